# revision 1
# baseline (speedup 1.0000x reference)
"""Multi-head causal attention (B=4, S=2048, D=1024, H=16) on 8 TRN2 cores.

Sharding: core = (batch, head-group): 4 batches x 2 groups of 8 heads.
Every core runs an identical program (uniform causal structure -> valid SPMD):
  - Q/K/V projections for its 8 heads over its batch's full 2048 rows
    (Q,K produced transposed [Dout, S]; V natural [S, Dout] + ones column)
  - causal attention per head-pair: scores_T = K_h @ Q_h^T via row-packed
    K=64 matmuls (tile_position); ACT exp straight from PSUM; multiplicative
    0/1 triangle mask on the diagonal 128-strip post-exp (DVE); attnV with
    M=65 (65th row accumulates the softmax denominator); reciprocal +
    gpsimd partition-broadcast; normalize into outT. The (pair, keytile)
    stream is software-pipelined (scores i+1 before attnV i) and projection/
    output-projection work for neighboring blocks is woven in as PE filler.
  - partial output projection y_part = outT^T @ Wo^T (contraction over this
    group's 512 channels)
Host: y[b] = y_part[b,g0] + y_part[b,g1] + bo.

All matmuls run as float32r (fp32 with 11-bit mantissa, ~tf32): inputs are
pre-rounded on host, intermediates rounded by the producing DVE/ACT op.
"""
import sys

for _p in ("/opt/trn_rl_repo", "/root/.axon_site/_ro/trn_rl_repo"):
    if _p not in sys.path:
        sys.path.append(_p)

import numpy as np
from contextlib import ExitStack

B, S, D, H = 4, 2048, 1024, 16
DK = D // H          # 64
HG = H // 2          # 8 heads per core
DG = HG * DK         # 512 channels per core
P = 128
NQB = S // 512       # 4 query blocks of 512
NKT = S // P         # 16 key tiles of 128
MASK_VAL = -1e5

_cache = {}


def _to_f32r(x):
    b = np.ascontiguousarray(x, dtype=np.float32).view(np.uint32)
    r = np.bitwise_and(b + np.uint32(0x800), np.uint32(0xFFFFF000))
    return r.view(np.float32)


def _build():
    import concourse.tile as tile
    from concourse import bacc, mybir

    f32 = mybir.dt.float32
    f32r = mybir.dt.float32r
    Exp = mybir.ActivationFunctionType.Exp

    nc = bacc.Bacc("TRN2", target_bir_lowering=False, debug=False,
                   enable_asserts=False, num_devices=8)

    xq = nc.dram_tensor("xq", [D, S], f32r, kind="ExternalInput").ap()
    xk = nc.dram_tensor("xk", [D, S], f32r, kind="ExternalInput").ap()
    xv = nc.dram_tensor("xv", [D, S], f32r, kind="ExternalInput").ap()
    wq = nc.dram_tensor("wq", [D, DG], f32r, kind="ExternalInput").ap()
    wk = nc.dram_tensor("wk", [D, DG], f32r, kind="ExternalInput").ap()
    wv = nc.dram_tensor("wv", [D, DG], f32r, kind="ExternalInput").ap()
    wo = nc.dram_tensor("wo", [DG, D], f32r, kind="ExternalInput").ap()
    bq = nc.dram_tensor("bq", [P, DG // P], f32, kind="ExternalInput").ap()
    bk = nc.dram_tensor("bk", [P, DG // P], f32, kind="ExternalInput").ap()
    bvr = nc.dram_tensor("bvr", [P, DG], f32, kind="ExternalInput").ap()
    masktri = nc.dram_tensor("masktri", [P, P], f32r, kind="ExternalInput").ap()
    onescol = nc.dram_tensor("onescol", [P, NKT, HG], f32r, kind="ExternalInput").ap()
    y = nc.dram_tensor("y", [S, D], f32, kind="ExternalOutput").ap()

    with tile.TileContext(nc) as tc, ExitStack() as ctx:
        persist = ctx.enter_context(tc.tile_pool(name="persist", bufs=1))
        consts = ctx.enter_context(tc.tile_pool(name="consts", bufs=1))
        xin_pool = ctx.enter_context(tc.tile_pool(name="xin", bufs=8))
        w_pool = ctx.enter_context(tc.tile_pool(name="w", bufs=6))
        wo_pool = ctx.enter_context(tc.tile_pool(name="wop", bufs=2))
        ot_pool = ctx.enter_context(tc.tile_pool(name="otpool", bufs=2))
        sb_small = ctx.enter_context(tc.tile_pool(name="sbs", bufs=2))
        y_pool = ctx.enter_context(tc.tile_pool(name="ysb", bufs=2))
        exp_pool = ctx.enter_context(tc.tile_pool(name="expp", bufs=4))
        # (bufs=4 exp made no difference; 3 leaves SBUF slack)

        # persistent intermediates, subtiled for fine-grained scheduling
        KTs = [[persist.tile([P, 512], f32r, tag=f"KT{m}_{qc}",
                             name=f"KT{m}_{qc}")
                for qc in range(4)] for m in range(4)]
        Vgs = [persist.tile([P, HG, DK + 1], f32r, tag=f"Vg{kt}",
                            name=f"Vg{kt}")
               for kt in range(NKT)]
        qt_pool = ctx.enter_context(tc.tile_pool(name="qtp", bufs=2))

        mask_t = consts.tile([P, P], f32r, tag="mask")
        bq_t = consts.tile([P, DG // P], f32, tag="bq")
        bk_t = consts.tile([P, DG // P], f32, tag="bk")
        bvr_t = consts.tile([P, DG], f32, tag="bvr")

        def load_xin(src, qb, eng=None):
            eng = eng or nc.sync
            xts = []
            for j in range(D // P):
                xt = xin_pool.tile([P, 512], f32r, tag="xin")
                eng.dma_start(
                    xt[:], src[j * P:(j + 1) * P, qb * 512:(qb + 1) * 512])
                xts.append(xt)
            return xts

        def load_w_halves(wsrc, eng=None):
            eng = eng or nc.sync
            whs = []
            for h2 in range(2):
                wh = w_pool.tile([P, D // P, DG // 2], f32r, tag="wh")
                eng.dma_start(
                    wh[:], wsrc[:, h2 * (DG // 2):(h2 + 1) * (DG // 2)]
                    .rearrange("(o p) m -> p o m", p=P))
                whs.append(wh)
            return whs

        sc_ps = ctx.enter_context(tc.tile_pool(name="scps", bufs=3, space="PSUM"))
        proj_ps = sc_ps
        oa_ps = ctx.enter_context(tc.tile_pool(name="oaps", bufs=1, space="PSUM"))

        # prefetch the exp table-set during the projection phase
        warm = consts.tile([1, 4], f32, tag="warm")
        nc.scalar.activation(warm[:], mask_t[0:1, 0:4], Exp)

        def proj_qk_m(whs, xts, bias_t, dst_tile, m):
            ps2 = proj_ps.tile([P, 2, 512], f32, tag="sc", name="pps")
            ps = ps2[:, 0, :]
            for j in range(D // P):
                nc.tensor.matmul(
                    ps[:], whs[m // 2][:, j, (m % 2) * P:(m % 2 + 1) * P],
                    xts[j][:], start=(j == 0), stop=(j == D // P - 1))
            nc.vector.tensor_scalar_add(
                dst_tile[:], ps[:], bias_t[:, m:m + 1])

        def proj_v_part(whs, xts, qb, mt_l, h2):
            mt = qb * 4 + mt_l
            ps2 = proj_ps.tile([P, 2, 512], f32, tag="sc", name="ppsv")
            ps = ps2[:, 0, :DG // 2]
            for j in range(D // P):
                nc.tensor.matmul(
                    ps[:], xts[j][:, mt_l * P:(mt_l + 1) * P],
                    whs[h2][:, j, :],
                    start=(j == 0), stop=(j == D // P - 1))
            nc.vector.tensor_add(
                Vgs[mt][:, h2 * 4:(h2 + 1) * 4, 0:DK],
                ps[:].rearrange("p (h d) -> p h d", h=4),
                bvr_t[:, h2 * 256:(h2 + 1) * 256]
                .rearrange("p (h d) -> p h d", h=4))

        def yproj_part(qb, OT, qt_l, nb):
            ps2 = sc_ps.tile([P, 2, 512], f32, tag="sc", name="yps")
            ps = ps2[:, 0, :]
            for t in range(DG // P):
                nc.tensor.matmul(
                    ps[:], OT[:, t, qt_l * P:(qt_l + 1) * P],
                    wos[nb][:, t, :],
                    start=(t == 0), stop=(t == DG // P - 1))
            ys = y_pool.tile([P, 512], f32, tag="ys", name="ys")
            nc.vector.tensor_copy(ys[:], ps[:])
            nc.sync.dma_start(
                y[(qb * 4 + qt_l) * P:(qb * 4 + qt_l + 1) * P,
                  nb * 512:(nb + 1) * 512],
                ys[:])

        def attention_qb(qb, QTcur, fillers):
            nfill0 = len(fillers)
            kmax = 4 * (qb + 1)
            nblk = (HG // 2) * kmax
            blk_done = [0]
            OT = ot_pool.tile([P, HG // 2, 512], f32r, tag="OT", name="OT")
            blocks = [(p, kt) for p in range(HG // 2) for kt in range(kmax)]
            oas_by_p = {}
            ex_by_blk = {}

            def emit_scores(p, kt):
                sc = sc_ps.tile([P, 2, 512], f32, tag="sc", name="sc")
                diag = kt >= 4 * qb
                f0 = (kt - 4 * qb) * P if diag else 0
                for hh in (0, 1):
                    nc.tensor.matmul(
                        sc[:, hh, f0:512],
                        KTs[p][kt // 4][hh * DK:(hh + 1) * DK,
                                        (kt % 4) * P:(kt % 4 + 1) * P],
                        QTcur[p][hh * DK:(hh + 1) * DK, f0:512],
                        start=True, stop=True,
                        tile_position=(hh * DK, 0))
                ex = exp_pool.tile([P, 2, 512], f32r, tag="ex", name="ex")
                nc.scalar.activation(ex[:, :, f0:512], sc[:, :, f0:512], Exp)
                if diag:
                    # causal mask, multiplicative post-exp: only the leading
                    # 128-col strip of the valid range is triangular
                    nc.vector.tensor_mul(
                        ex[:, :, f0:f0 + P],
                        ex[:, :, f0:f0 + P],
                        mask_t[:, None, 0:P].to_broadcast((P, 2, P)))
                ex_by_blk[(p, kt)] = ex

            def emit_attnv(p, kt):
                diag = kt >= 4 * qb
                f0 = (kt - 4 * qb) * P if diag else 0
                if kt == 0:
                    oa0 = oa_ps.tile([DK + 1, 512], f32, tag="oa0",
                                     name="oa0")
                    oa1 = oa_ps.tile([DK + 1, 512], f32, tag="oa1",
                                     name="oa1")
                    oas_by_p[p] = (oa0, oa1)
                oas = oas_by_p[p]
                ex = ex_by_blk.pop((p, kt))
                for hh in (0, 1):
                    nc.tensor.matmul(
                        oas[hh][:, f0:512],
                        Vgs[kt][:, 2 * p + hh, :],
                        ex[:, hh, f0:512],
                        start=(kt == 0), stop=(kt == kmax - 1))
                if kt == kmax - 1:
                    for hh in (0, 1):
                        rs = sb_small.tile([1, 512], f32, tag="rs", name="rs")
                        nc.vector.reciprocal(rs[:], oas[hh][DK:DK + 1, :])
                        rb_sb = sb_small.tile([DK, 512], f32, tag="rbsb",
                                              name="rbsb")
                        nc.gpsimd.partition_broadcast(rb_sb[:], rs[:])
                        nc.vector.tensor_mul(
                            OT[hh * DK:(hh + 1) * DK, p, :],
                            oas[hh][0:DK, :], rb_sb[:])
                # weave filler work evenly across all kt blocks
                blk_done[0] += 1
                target_done = (nfill0 * blk_done[0]) // nblk
                while fillers and nfill0 - len(fillers) < target_done:
                    fillers.pop(0)()

            # software pipeline: scores(i+1) issued before attnV(i)
            emit_scores(*blocks[0])
            for i in range(len(blocks)):
                if i + 1 < len(blocks):
                    emit_scores(*blocks[i + 1])
                emit_attnv(*blocks[i])
            for f in fillers:
                f()
            del fillers[:]
            return OT

        def proj_block(qb):
            """Emit projections for block qb; returns (QTcur, filler list)."""
            fillers = []
            xtsv = load_xin(xv, qb)
            for mt_l in range(4):
                for h2 in range(2):
                    fillers.append(
                        lambda mt_l=mt_l, h2=h2, xtsv=xtsv, qb=qb:
                        proj_v_part(wv_hs, xtsv, qb, mt_l, h2))
            QTcur = []
            for m in range(DG // P):
                qt_t = qt_pool.tile([P, 512], f32r, tag=f"QTm{m}",
                                    name=f"QTm{m}")
                QTcur.append(qt_t)
            xtsq = load_xin(xq, qb)
            for m in range(DG // P):
                fillers.append(
                    lambda m=m, xtsq=xtsq: proj_qk_m(
                        wq_hs, xtsq, bq_t, QTcur[m], m))
            xts = load_xin(xk, qb)
            for m in range(DG // P):
                fillers.append(
                    lambda m=m, xts=xts, qb=qb:
                    proj_qk_m(wk_hs, xts, bk_t, KTs[m][qb], m))
            return QTcur, fillers

        # block 0: interleave weight/const loads with their first users
        wk_hs = load_w_halves(wk)
        nc.sync.dma_start(bk_t[:], bk)
        nc.sync.dma_start(bq_t[:], bq)
        xts = load_xin(xk, 0)
        for m in range(DG // P):
            proj_qk_m(wk_hs, xts, bk_t, KTs[m][0], m)
        wv_hs = load_w_halves(wv)
        nc.sync.dma_start(bvr_t[:], bvr)
        for kt in range(NKT):
            nc.sync.dma_start(Vgs[kt][:, :, DK], onescol[:, kt, :])
        xtsv = load_xin(xv, 0)
        for mt_l in range(4):
            for h2 in range(2):
                proj_v_part(wv_hs, xtsv, 0, mt_l, h2)
        wq_hs = load_w_halves(wq)
        nc.sync.dma_start(mask_t[:], masktri)
        QTcur = []
        for m in range(DG // P):
            qt_t = qt_pool.tile([P, 512], f32r, tag=f"QTm{m}", name=f"QTm{m}")
            QTcur.append(qt_t)
        xtsq = load_xin(xq, 0)
        for m in range(DG // P):
            proj_qk_m(wq_hs, xtsq, bq_t, QTcur[m], m)
        wos = []
        for nb in range(2):                   # [DG, D] -> halves [128,4,512]
            wh = wo_pool.tile([P, DG // P, 512], f32r, tag="wo", name="wo")
            nc.sync.dma_start(
                wh[:], wo[:, nb * 512:(nb + 1) * 512]
                .rearrange("(o p) m -> p o m", p=P))
            wos.append(wh)

        prevOT = None
        for qb in range(NQB):
            fillers = []
            if prevOT is not None:
                for qt_l in range(4):
                    for nb in range(2):
                        fillers.append(
                            lambda qt_l=qt_l, nb=nb, O=prevOT, q=qb - 1:
                            yproj_part(q, O, qt_l, nb))
            nextQT = None
            if qb + 1 < NQB:
                nextQT, pf = proj_block(qb + 1)
                fillers.extend(pf)
            prevOT = attention_qb(qb, QTcur, fillers)
            QTcur = nextQT
        for qt_l in range(4):
            for nb in range(2):
                yproj_part(NQB - 1, prevOT, qt_l, nb)

    nc.compile()
    return nc


def _prep_inputs(query, key, value, Wq, bq, Wk, bk, Wv, bv, Wo, bo):
    scale = 1.0 / np.sqrt(DK)
    qr = _to_f32r(np.asarray(query))
    kr = _to_f32r(np.asarray(key))
    vr = _to_f32r(np.asarray(value))
    wq_full = _to_f32r(np.asarray(Wq).T * scale)   # [D, D], cols = out chans
    wk_full = _to_f32r(np.asarray(Wk).T)
    wv_full = _to_f32r(np.asarray(Wv).T)
    wo_full = _to_f32r(np.asarray(Wo).T)           # [Din, Dout]
    bq_s = np.asarray(bq) * scale

    # constant tensors
    jj = np.arange(P)[:, None]
    ff = np.arange(P)[None, :]
    masktri = np.where(jj <= ff, 1.0, 0.0).astype(np.float32)
    onescol = np.ones((P, NKT, HG), np.float32)

    in_maps = []
    for core in range(8):
        b, hg = core // 2, core % 2
        sl = slice(hg * DG, (hg + 1) * DG)
        in_maps.append({
            "xq": np.ascontiguousarray(qr[b].T),
            "xk": np.ascontiguousarray(kr[b].T),
            "xv": np.ascontiguousarray(vr[b].T),
            "wq": np.ascontiguousarray(wq_full[:, sl]),
            "wk": np.ascontiguousarray(wk_full[:, sl]),
            "wv": np.ascontiguousarray(wv_full[:, sl]),
            "wo": np.ascontiguousarray(wo_full[sl, :]),
            "bq": np.ascontiguousarray(
                bq_s[sl].reshape(DG // P, P).T.astype(np.float32)),
            "bk": np.ascontiguousarray(
                np.asarray(bk)[sl].reshape(DG // P, P).T.astype(np.float32)),
            "bvr": np.broadcast_to(
                np.asarray(bv)[sl].astype(np.float32), (P, DG)).copy(),
            "masktri": masktri,
            "onescol": onescol,
        })
    return in_maps


def kernel(query, key, value, mask, Wq, bq, Wk, bk, Wv, bv, Wo, bo,
           **run_kwargs):
    from concourse.bass_utils import run_bass_kernel_spmd

    if "nc" not in _cache:
        _cache["nc"] = _build()
    nc = _cache["nc"]

    in_maps = _prep_inputs(query, key, value, Wq, bq, Wk, bk, Wv, bv, Wo, bo)
    res = run_bass_kernel_spmd(nc, in_maps, core_ids=list(range(8)),
                               **run_kwargs)
    bo = np.asarray(bo, dtype=np.float32)
    out = np.empty((B, S, D), dtype=np.float32)
    for b in range(B):
        out[b] = res.results[2 * b]["y"] + res.results[2 * b + 1]["y"] + bo
    _cache["last_results"] = res
    return out



# revision 14
# speedup vs baseline: 1.2338x; 1.2338x over previous
"""Multi-head causal attention (B=4, S=2048, D=1024, H=16) on 8 TRN2 cores.

Sharding: core = (batch, head-group): 4 batches x 2 groups of 8 heads.
Every core runs an identical program (uniform causal structure -> valid SPMD):
  - Q/K/V projections for its 8 heads over its batch's full 2048 rows
    (Q,K produced transposed [Dout, S]; V natural [S, Dout] + ones column)
  - causal attention per (pair, keytile, head) unit: scores_T = K_h @ Q_h^T,
    ACT exp straight from PSUM into bf16; multiplicative 0/1 triangle mask on
    the diagonal 128-strip post-exp (DVE); attnV with M=65 (65th row
    accumulates the softmax denominator); reciprocal + gpsimd
    partition-broadcast; normalize into OT. Units are software-pipelined
    (scores i+DEPTH issued before attnV i) with projection/output-projection
    work woven in as PE filler under deadline constraints.
  - partial output projection y_part = OT^T @ Wo^T (contraction over this
    group's 512 channels)
Host: y[b] = y_part[b,g0] + y_part[b,g1] + bo.

All matmuls run in bf16 (inputs/weights rounded on host; intermediates
rounded by the producing DVE/ACT op); PSUM accumulation is fp32.
"""
import sys

for _p in ("/opt/trn_rl_repo", "/root/.axon_site/_ro/trn_rl_repo"):
    if _p not in sys.path:
        sys.path.append(_p)

import numpy as np
from contextlib import ExitStack

B, S, D, H = 4, 2048, 1024, 16
DK = D // H          # 64
HG = H // 2          # 8 heads per core
DG = HG * DK         # 512 channels per core
P = 128
NQB = S // 512       # 4 query blocks of 512
NKT = S // P         # 16 key tiles of 128
PIPE_DEPTH = 3

_cache = {}


def _to_bf16(x):
    import ml_dtypes
    return np.ascontiguousarray(np.asarray(x, dtype=np.float32)).astype(
        ml_dtypes.bfloat16)


def _build():
    import concourse.tile as tile
    from concourse import bacc, mybir

    f32 = mybir.dt.float32
    bf16 = mybir.dt.bfloat16
    Exp = mybir.ActivationFunctionType.Exp

    nc = bacc.Bacc("TRN2", target_bir_lowering=False, debug=False,
                   enable_asserts=False, num_devices=8)

    xq = nc.dram_tensor("xq", [D, S], bf16, kind="ExternalInput").ap()
    xk = nc.dram_tensor("xk", [D, S], bf16, kind="ExternalInput").ap()
    xv = nc.dram_tensor("xv", [D, S], bf16, kind="ExternalInput").ap()
    wq = nc.dram_tensor("wq", [D, DG], bf16, kind="ExternalInput").ap()
    wk = nc.dram_tensor("wk", [D, DG], bf16, kind="ExternalInput").ap()
    wv = nc.dram_tensor("wv", [D, DG], bf16, kind="ExternalInput").ap()
    wo = nc.dram_tensor("wo", [DG, D], bf16, kind="ExternalInput").ap()
    bq = nc.dram_tensor("bq", [P, DG // P], f32, kind="ExternalInput").ap()
    bk = nc.dram_tensor("bk", [P, DG // P], f32, kind="ExternalInput").ap()
    bvr = nc.dram_tensor("bvr", [P, HG, DK], f32, kind="ExternalInput").ap()
    masktri = nc.dram_tensor("masktri", [P, P], bf16, kind="ExternalInput").ap()
    y = nc.dram_tensor("y", [S, D], bf16, kind="ExternalOutput").ap()

    with tile.TileContext(nc) as tc, ExitStack() as ctx:
        persist = ctx.enter_context(tc.tile_pool(name="persist", bufs=1))
        consts = ctx.enter_context(tc.tile_pool(name="consts", bufs=1))
        xin_pool = ctx.enter_context(tc.tile_pool(name="xin", bufs=6))
        qt_pool = ctx.enter_context(tc.tile_pool(name="qtp", bufs=2))
        ot_pool = ctx.enter_context(tc.tile_pool(name="otpool", bufs=3))
        sb_small = ctx.enter_context(tc.tile_pool(name="sbs", bufs=2))
        y_pool = ctx.enter_context(tc.tile_pool(name="ysb", bufs=3))
        exp_pool = ctx.enter_context(tc.tile_pool(name="expp", bufs=6))

        # persistent weights / K / V
        wq_t = persist.tile([P, D // P, DG], bf16, tag="wq", name="wq")
        wk_t = persist.tile([P, D // P, DG], bf16, tag="wk", name="wk")
        wv_t = persist.tile([P, D // P, DG], bf16, tag="wv", name="wv")
        wo_t = persist.tile([P, DG // P, D], bf16, tag="wo", name="wo")
        KTs = [[persist.tile([P, 512], bf16, tag=f"KT{m}_{qc}",
                             name=f"KT{m}_{qc}")
                for qc in range(4)] for m in range(4)]
        Vg = persist.tile([P, NKT, HG, DK + 1], bf16, tag="Vg", name="Vg")

        mask_t = consts.tile([P, P], bf16, tag="mask")
        bq_t = consts.tile([P, DG // P], f32, tag="bq")
        bk_t = consts.tile([P, DG // P], f32, tag="bk")
        bvr_t = consts.tile([P, HG, DK], f32, tag="bvr")
        warm_in = consts.tile([1, 4], f32, tag="warmin")
        warm = consts.tile([1, 4], f32, tag="warm")

        # PSUM pools: 4 (scores) + 2 (attnV accum) + 1 (proj) + 1 (yproj)
        sc_ps = ctx.enter_context(tc.tile_pool(name="scps", bufs=PIPE_DEPTH + 1,
                                               space="PSUM"))
        oa_ps = ctx.enter_context(tc.tile_pool(name="oaps", bufs=1, space="PSUM"))
        pp_ps = ctx.enter_context(tc.tile_pool(name="ppps", bufs=1, space="PSUM"))
        yp_ps = ctx.enter_context(tc.tile_pool(name="ypps", bufs=1, space="PSUM"))

        # prefetch the exp table-set before any real work
        nc.gpsimd.memset(warm_in[:], 0.0)
        nc.scalar.activation(warm[:], warm_in[:], Exp)
        # ones column of V (softmax denominator trick)
        nc.gpsimd.memset(Vg[:, :, :, DK], 1.0)

        def load_xin(src, qb, splits=1):
            xt = xin_pool.tile([P, D // P, 512], bf16, tag="xin")
            step = (D // P) // splits
            for s in range(splits):
                r0, r1 = s * step * P, (s + 1) * step * P
                nc.sync.dma_start(
                    xt[:, s * step:(s + 1) * step, :],
                    src[r0:r1, qb * 512:(qb + 1) * 512]
                    .rearrange("(o p) m -> p o m", p=P))
            return xt

        # alternate proj PSUM between the pp and yp banks to avoid WAR chains
        alt_state = [0]

        def proj_ps_tile():
            alt_state[0] ^= 1
            pool, tag = ((pp_ps, "pp"), (yp_ps, "yp"))[alt_state[0]]
            return pool.tile([P, 512], f32, tag=tag, name=tag)

        def proj_qk_m(w_t, xt, bias_t, dst_tile, m):
            ps = proj_ps_tile()
            for j in range(D // P):
                nc.tensor.matmul(
                    ps[:], w_t[:, j, m * P:(m + 1) * P], xt[:, j, :],
                    start=(j == 0), stop=(j == D // P - 1))
            nc.vector.tensor_scalar_add(
                dst_tile[:], ps[:], bias_t[:, m:m + 1])

        def proj_v_part(xt, qb, mt_l, h2):
            kt = qb * 4 + mt_l
            ps = proj_ps_tile()
            psv = ps[:, 0:DG // 2]
            for j in range(D // P):
                nc.tensor.matmul(
                    psv, xt[:, j, mt_l * P:(mt_l + 1) * P],
                    wv_t[:, j, h2 * 256:(h2 + 1) * 256],
                    start=(j == 0), stop=(j == D // P - 1))
            nc.vector.tensor_add(
                Vg[:, kt, h2 * 4:(h2 + 1) * 4, 0:DK],
                psv.rearrange("p (h d) -> p h d", h=4),
                bvr_t[:, h2 * 4:(h2 + 1) * 4, :])

        ycopy_state = [0]

        def yproj_part(qb, OT, qt_l, nb, ps_pool, ps_tag, split_t=False,
                       tail_ys=None):
            """Emit output-projection part. If split_t, t=0..2 are emitted by
            the caller via the returned closure pair (prefill, finish)."""
            ps = ps_pool.tile([P, 512], f32, tag=ps_tag, name="yp")

            def mm(t):
                nc.tensor.matmul(
                    ps[:], OT[:, t, qt_l * P:(qt_l + 1) * P],
                    wo_t[:, t, nb * 512:(nb + 1) * 512],
                    start=(t == 0), stop=(t == DG // P - 1))

            def finish():
                mm(DG // P - 1)
                if tail_ys is not None:
                    # tail: alternate DVE / ACT so copies pipeline 2-wide
                    # (gpsimd cannot read PSUM)
                    ys_t, idx = tail_ys
                    ycopy_state[0] ^= 1
                    if ycopy_state[0]:
                        nc.vector.tensor_copy(ys_t[:, idx, :], ps[:])
                    else:
                        nc.scalar.activation(
                            ys_t[:, idx, :], ps[:],
                            mybir.ActivationFunctionType.Copy)
                else:
                    ys = y_pool.tile([P, 512], bf16, tag="ys", name="ys")
                    nc.vector.tensor_copy(ys[:], ps[:])
                    nc.sync.dma_start(
                        y[(qb * 4 + qt_l) * P:(qb * 4 + qt_l + 1) * P,
                          nb * 512:(nb + 1) * 512],
                        ys[:])

            def prefill():
                for t in range(DG // P - 1):
                    mm(t)

            if split_t:
                return prefill, finish
            prefill()
            finish()

        def attention_qb(qb, QTcur, fillers, late_fillers=()):
            """fillers: evenly woven closures. late_fillers: (deadline_unit,
            closure) — emitted once done-count reaches deadline (deadlines
            must be achievable: closure emitted before its consumer unit)."""
            nfill0 = len(fillers)
            kmax = 4 * (qb + 1)
            units = [(p, kt, hh)
                     for p in range(HG // 2) for kt in range(kmax)
                     for hh in (0, 1)]
            nunits = len(units)
            done = [0]
            late = list(late_fillers)
            OT = ot_pool.tile([P, HG // 2, 512], bf16, tag="OT", name="OT")
            oas = {}
            ex_by = {}

            def emit_scores(p, kt, hh):
                diag = kt >= 4 * qb
                f0 = (kt - 4 * qb) * P if diag else 0
                sc = sc_ps.tile([P, 512], f32, tag="sc", name="sc")
                nc.tensor.matmul(
                    sc[:, f0:512],
                    KTs[p][kt // 4][hh * DK:(hh + 1) * DK,
                                    (kt % 4) * P:(kt % 4 + 1) * P],
                    QTcur[p][hh * DK:(hh + 1) * DK, f0:512],
                    start=True, stop=True, tile_position=(hh * DK, 0))
                ex = exp_pool.tile([P, 512], bf16, tag="ex", name="ex")
                nc.scalar.activation(ex[:, f0:512], sc[:, f0:512], Exp)
                if diag:
                    # causal mask, multiplicative post-exp: only the leading
                    # 128-col strip of the valid range is triangular
                    nc.vector.tensor_mul(
                        ex[:, f0:f0 + P], ex[:, f0:f0 + P], mask_t[:, 0:P])
                ex_by[(p, kt, hh)] = ex

            def emit_attnv(p, kt, hh):
                diag = kt >= 4 * qb
                f0 = (kt - 4 * qb) * P if diag else 0
                if kt == 0:
                    oas[(p, hh)] = oa_ps.tile([DK + 1, 512], f32,
                                              tag=f"oa{hh}", name=f"oa{hh}")
                oa = oas[(p, hh)]
                ex = ex_by.pop((p, kt, hh))
                nc.tensor.matmul(
                    oa[:, f0:512], Vg[:, kt, 2 * p + hh, :], ex[:, f0:512],
                    start=(kt == 0), stop=(kt == kmax - 1))
                nf = 0
                if kt == kmax - 1:
                    rs = sb_small.tile([1, 512], f32, tag="rs", name="rs")
                    nc.vector.reciprocal(rs[:], oa[DK:DK + 1, :])
                    rb = sb_small.tile([DK, 512], f32, tag="rb", name="rb")
                    nc.gpsimd.partition_broadcast(rb[:], rs[:])
                    nc.vector.tensor_mul(
                        OT[hh * DK:(hh + 1) * DK, p, :], oa[0:DK, :], rb[:])
                    if hh == 0 and p + 1 < HG // 2:
                        # pair boundary: next pair's attnV kt=0 must wait for
                        # this pair's oa reads; pull extra filler cover here
                        nf = 2
                done[0] += 1
                while late and late[0][0] <= done[0]:
                    late.pop(0)[1]()
                target_done = (nfill0 * done[0]) // nunits
                while fillers and (nfill0 - len(fillers) < target_done + nf):
                    fillers.pop(0)()
                    if nf:
                        nf -= 1

            dd = min(PIPE_DEPTH, nunits)
            for j in range(dd):
                emit_scores(*units[j])
            for i in range(nunits):
                if i + dd < nunits:
                    emit_scores(*units[i + dd])
                emit_attnv(*units[i])
            while late:
                late.pop(0)[1]()
            for f in fillers:
                f()
            del fillers[:]
            return OT

        def proj_block(qb):
            """Emit projections for block qb. Returns (QTcur, fillers,
            late_fillers): late ones run inside attention_qb(qb) itself,
            before their consumer pairs start."""
            fillers = []
            xtv = load_xin(xv, qb)
            for mt_l in range(4):
                for h2 in range(2):
                    fillers.append(
                        lambda mt_l=mt_l, h2=h2, xtv=xtv, qb=qb:
                        proj_v_part(xtv, qb, mt_l, h2))
            QTcur = []
            for m in range(DG // P):
                qt_t = qt_pool.tile([P, 512], bf16, tag=f"QTm{m}",
                                    name=f"QTm{m}")
                QTcur.append(qt_t)
            xtq = load_xin(xq, qb)
            for m in range(2):
                fillers.append(
                    lambda m=m, xtq=xtq: proj_qk_m(
                        wq_t, xtq, bq_t, QTcur[m], m))
            xtk = load_xin(xk, qb)
            for m in range(2):
                fillers.append(
                    lambda m=m, xtk=xtk, qb=qb:
                    proj_qk_m(wk_t, xtk, bk_t, KTs[m][qb], m))
            # pairs 2 and 3 of block qb are consumed late inside
            # attention_qb(qb): project them there (deadline = unit index
            # safely before first consumer emission p*2*kmax - DEPTH)
            kmax_n = 4 * (qb + 1)
            late = []
            for m in (2, 3):
                dl = max(1, m * 2 * kmax_n - PIPE_DEPTH - 4)
                late.append((dl - 8, lambda m=m, xtq=xtq: proj_qk_m(
                    wq_t, xtq, bq_t, QTcur[m], m)))
                late.append((dl, lambda m=m, xtk=xtk, qb=qb: proj_qk_m(
                    wk_t, xtk, bk_t, KTs[m][qb], m)))
            late.sort(key=lambda t: t[0])
            return QTcur, fillers, late

        # ---- block 0 prologue: interleave loads with their first users ----
        nc.sync.dma_start(wk_t[:, :, 0:256],
                          wk[:, 0:256].rearrange("(o p) m -> p o m", p=P))
        xtk0 = load_xin(xk, 0, splits=4)
        nc.sync.dma_start(bk_t[:], bk)
        nc.sync.dma_start(wk_t[:, :, 256:512],
                          wk[:, 256:512].rearrange("(o p) m -> p o m", p=P))
        for m in range(DG // P):
            proj_qk_m(wk_t, xtk0, bk_t, KTs[m][0], m)
        xtv0 = load_xin(xv, 0, splits=2)
        nc.sync.dma_start(wv_t[:, :, 0:256],
                          wv[:, 0:256].rearrange("(o p) m -> p o m", p=P))
        nc.sync.dma_start(wv_t[:, :, 256:512],
                          wv[:, 256:512].rearrange("(o p) m -> p o m", p=P))
        nc.sync.dma_start(bvr_t[:], bvr)
        for mt_l in range(4):
            for h2 in range(2):
                proj_v_part(xtv0, 0, mt_l, h2)
        xtq0 = load_xin(xq, 0, splits=2)
        nc.sync.dma_start(wq_t[:, :, 0:256],
                          wq[:, 0:256].rearrange("(o p) m -> p o m", p=P))
        nc.sync.dma_start(wq_t[:, :, 256:512],
                          wq[:, 256:512].rearrange("(o p) m -> p o m", p=P))
        nc.sync.dma_start(bq_t[:], bq)
        nc.sync.dma_start(mask_t[:], masktri)
        QTcur = []
        for m in range(DG // P):
            qt_t = qt_pool.tile([P, 512], bf16, tag=f"QTm{m}", name=f"QTm{m}")
            QTcur.append(qt_t)
        for m in range(DG // P):
            proj_qk_m(wq_t, xtq0, bq_t, QTcur[m], m)
        nc.sync.dma_start(wo_t[:], wo.rearrange("(o p) m -> p o m", p=P))

        def yproj_filler(q, O, part_i):
            qt_l, nb = part_i // 2, part_i % 2
            pool, tag = ((pp_ps, "pp"), (yp_ps, "yp"))[part_i % 2]
            return (lambda qt_l=qt_l, nb=nb, O=O, q=q, pool=pool, tag=tag:
                    yproj_part(q, O, qt_l, nb, pool, tag))

        prevOT = None     # OT of qb-1
        prev2OT = None    # OT of qb-2 (second half of its yproj deferred)
        pending_late = ()
        for qb in range(NQB):
            last = qb == NQB - 1
            fillers = []
            if prev2OT is not None:
                for part_i in range(4, 8):
                    fillers.append(yproj_filler(qb - 2, prev2OT, part_i))
            if prevOT is not None:
                nparts = 8 if last else 4
                for part_i in range(nparts):
                    fillers.append(yproj_filler(qb - 1, prevOT, part_i))
            nextQT = None
            late = ()
            if not last:
                nextQT, pf, late = proj_block(qb + 1)
                fillers.extend(pf)
            OT = attention_qb(qb, QTcur, fillers, late_fillers=pending_late)
            pending_late = late
            prev2OT, prevOT = prevOT, OT
            QTcur = nextQT

        # ---- tail: output projection of the last block, t-split across 4
        # PSUM banks (yp, pp and two sc rotations are all free now);
        # y stores grouped into two wide DMAs ----
        ys_tail = persist.tile([P, 8, 512], bf16, tag="ystail", name="ystail")
        tailpools = [(yp_ps, "yp"), (pp_ps, "pp"), (sc_ps, "sc"), (sc_ps, "sc")]
        parts = [(qt_l, nb) for qt_l in range(4) for nb in range(2)]
        pf_fin = []
        for i, (qt_l, nb) in enumerate(parts):
            pool, tag = tailpools[i % 4]
            pf, fin = yproj_part(NQB - 1, prevOT, qt_l, nb, pool, tag,
                                 split_t=True, tail_ys=(ys_tail, i))
            pf_fin.append((pf, fin))
        # prefill t=0..2 of the first 4 parts (only needs pairs 0..2 of OT),
        # then stream finishes; later parts prefill as their bank frees
        for i in range(4):
            pf_fin[i][0]()
        for i in range(len(parts)):
            pf_fin[i][1]()
            if i + 4 < len(parts):
                pf_fin[i + 4][0]()
            if i == 3:
                nc.sync.dma_start(
                    y[(NQB - 1) * 512:(NQB - 1) * 512 + 256, :]
                    .rearrange("(q p) (n m) -> p q n m", p=P, n=2),
                    ys_tail[:, 0:4, :]
                    .rearrange("p (q n) m -> p q n m", q=2))
            if i == 7:
                nc.sync.dma_start(
                    y[(NQB - 1) * 512 + 256:NQB * 512, :]
                    .rearrange("(q p) (n m) -> p q n m", p=P, n=2),
                    ys_tail[:, 4:8, :]
                    .rearrange("p (q n) m -> p q n m", q=2))

    nc.compile()
    return nc


def _prep_inputs(query, key, value, Wq, bq, Wk, bk, Wv, bv, Wo, bo):
    scale = 1.0 / np.sqrt(DK)
    qr = _to_bf16(np.asarray(query))
    kr = _to_bf16(np.asarray(key))
    vr = _to_bf16(np.asarray(value))
    wq_full = _to_bf16(np.asarray(Wq).T * scale)   # [D, D], cols = out chans
    wk_full = _to_bf16(np.asarray(Wk).T)
    wv_full = _to_bf16(np.asarray(Wv).T)
    wo_full = _to_bf16(np.asarray(Wo).T)           # [Din, Dout]
    bq_s = np.asarray(bq) * scale

    jj = np.arange(P)[:, None]
    ff = np.arange(P)[None, :]
    masktri = _to_bf16(np.where(jj <= ff, 1.0, 0.0))

    in_maps = []
    for core in range(8):
        b, hg = core // 2, core % 2
        sl = slice(hg * DG, (hg + 1) * DG)
        in_maps.append({
            "xq": np.ascontiguousarray(qr[b].T),
            "xk": np.ascontiguousarray(kr[b].T),
            "xv": np.ascontiguousarray(vr[b].T),
            "wq": np.ascontiguousarray(wq_full[:, sl]),
            "wk": np.ascontiguousarray(wk_full[:, sl]),
            "wv": np.ascontiguousarray(wv_full[:, sl]),
            "wo": np.ascontiguousarray(wo_full[sl, :]),
            "bq": np.ascontiguousarray(
                bq_s[sl].reshape(DG // P, P).T.astype(np.float32)),
            "bk": np.ascontiguousarray(
                np.asarray(bk)[sl].reshape(DG // P, P).T.astype(np.float32)),
            "bvr": np.broadcast_to(
                np.asarray(bv)[sl].astype(np.float32).reshape(HG, DK),
                (P, HG, DK)).copy(),
            "masktri": masktri,
        })
    return in_maps


def kernel(query, key, value, mask, Wq, bq, Wk, bk, Wv, bv, Wo, bo,
           **run_kwargs):
    from concourse.bass_utils import run_bass_kernel_spmd

    if "nc" not in _cache:
        _cache["nc"] = _build()
    nc = _cache["nc"]

    in_maps = _prep_inputs(query, key, value, Wq, bq, Wk, bk, Wv, bv, Wo, bo)
    res = run_bass_kernel_spmd(nc, in_maps, core_ids=list(range(8)),
                               **run_kwargs)
    bo = np.asarray(bo, dtype=np.float32)
    out = np.empty((B, S, D), dtype=np.float32)
    for b in range(B):
        out[b] = (res.results[2 * b]["y"].astype(np.float32)
                  + res.results[2 * b + 1]["y"].astype(np.float32) + bo)
    _cache["last_results"] = res
    return out


# revision 17
# speedup vs baseline: 1.2855x; 1.0419x over previous
"""Multi-head causal attention (B=4, S=2048, D=1024, H=16) on 8 TRN2 cores.

Sharding: core = (batch, head-group): 4 batches x 2 groups of 8 heads.
Every core runs an identical program (uniform causal structure -> valid SPMD):
  - Q/K/V projections for its 8 heads over its batch's full 2048 rows
    (Q,K produced transposed [Dout, S]; V natural [S, Dout] + ones column)
  - causal attention per (pair, keytile, head) unit: scores_T = K_h @ Q_h^T,
    ACT exp straight from PSUM into bf16; multiplicative 0/1 triangle mask on
    the diagonal 128-strip post-exp (DVE); attnV with M=65 (65th row
    accumulates the softmax denominator); reciprocal + gpsimd
    partition-broadcast; normalize into OT. Units are software-pipelined
    (scores i+DEPTH issued before attnV i) with projection/output-projection
    work woven in as PE filler under deadline constraints.
  - partial output projection y_part = OT^T @ Wo^T (contraction over this
    group's 512 channels)
Host: y[b] = y_part[b,g0] + y_part[b,g1] + bo.

All matmuls run in bf16 (inputs/weights rounded on host; intermediates
rounded by the producing DVE/ACT op); PSUM accumulation is fp32.
"""
import sys

for _p in ("/opt/trn_rl_repo", "/root/.axon_site/_ro/trn_rl_repo"):
    if _p not in sys.path:
        sys.path.append(_p)

import numpy as np
from contextlib import ExitStack

B, S, D, H = 4, 2048, 1024, 16
DK = D // H          # 64
HG = H // 2          # 8 heads per core
DG = HG * DK         # 512 channels per core
P = 128
NQB = S // 512       # 4 query blocks of 512
NKT = S // P         # 16 key tiles of 128

# PSUM-bank/pipeline configuration (8 banks total):
#   sc = SC_BUFS, oa = 2*OA_BUFS, proj = 1, yproj = SEP_YP
import os
PIPE_DEPTH = int(os.environ.get("K_DEPTH", "3"))
OA_BUFS = int(os.environ.get("K_OA", "1"))
SEP_YP = int(os.environ.get("K_SEPYP", "1"))
SC_BUFS = 8 - 2 * OA_BUFS - 1 - SEP_YP

_cache = {}


def _to_bf16(x):
    import ml_dtypes
    return np.ascontiguousarray(np.asarray(x, dtype=np.float32)).astype(
        ml_dtypes.bfloat16)


def _build():
    import concourse.tile as tile
    from concourse import bacc, mybir

    f32 = mybir.dt.float32
    bf16 = mybir.dt.bfloat16
    Exp = mybir.ActivationFunctionType.Exp

    nc = bacc.Bacc("TRN2", target_bir_lowering=False, debug=False,
                   enable_asserts=False, num_devices=8)

    xq = nc.dram_tensor("xq", [D, S], bf16, kind="ExternalInput").ap()
    xk = nc.dram_tensor("xk", [D, S], bf16, kind="ExternalInput").ap()
    xv = nc.dram_tensor("xv", [D, S], bf16, kind="ExternalInput").ap()
    wq = nc.dram_tensor("wq", [D, DG], bf16, kind="ExternalInput").ap()
    wk = nc.dram_tensor("wk", [D, DG], bf16, kind="ExternalInput").ap()
    wv = nc.dram_tensor("wv", [D, DG], bf16, kind="ExternalInput").ap()
    wo = nc.dram_tensor("wo", [DG, D], bf16, kind="ExternalInput").ap()
    bq = nc.dram_tensor("bq", [P, DG // P], f32, kind="ExternalInput").ap()
    bk = nc.dram_tensor("bk", [P, DG // P], f32, kind="ExternalInput").ap()
    bvr = nc.dram_tensor("bvr", [P, HG, DK], f32, kind="ExternalInput").ap()
    masktri = nc.dram_tensor("masktri", [P, P], bf16, kind="ExternalInput").ap()
    ident = nc.dram_tensor("ident", [P, P], bf16, kind="ExternalInput").ap()
    y = nc.dram_tensor("y", [S, D], bf16, kind="ExternalOutput").ap()

    with tile.TileContext(nc) as tc, ExitStack() as ctx:
        persist = ctx.enter_context(tc.tile_pool(name="persist", bufs=1))
        consts = ctx.enter_context(tc.tile_pool(name="consts", bufs=1))
        xin_pool = ctx.enter_context(tc.tile_pool(name="xin", bufs=6))
        qt_pool = ctx.enter_context(tc.tile_pool(name="qtp", bufs=2))
        ot_pool = ctx.enter_context(tc.tile_pool(name="otpool", bufs=3))
        sb_small = ctx.enter_context(tc.tile_pool(name="sbs", bufs=2))
        outn_pool = ctx.enter_context(tc.tile_pool(name="outn", bufs=2))
        y_pool = ctx.enter_context(tc.tile_pool(name="ysb", bufs=3))
        exp_pool = ctx.enter_context(tc.tile_pool(name="expp", bufs=6))

        # persistent weights / K / V
        wq_t = persist.tile([P, D // P, DG], bf16, tag="wq", name="wq")
        wk_t = persist.tile([P, D // P, DG], bf16, tag="wk", name="wk")
        wv_t = persist.tile([P, D // P, DG], bf16, tag="wv", name="wv")
        wo_t = persist.tile([P, DG // P, D], bf16, tag="wo", name="wo")
        KTs = [[persist.tile([P, 512], bf16, tag=f"KT{m}_{qc}",
                             name=f"KT{m}_{qc}")
                for qc in range(4)] for m in range(4)]
        Vg = persist.tile([P, NKT, HG, DK + 1], bf16, tag="Vg", name="Vg")

        mask_t = consts.tile([P, P], bf16, tag="mask")
        ident_t = consts.tile([P, P], bf16, tag="ident")
        bq_t = consts.tile([P, DG // P], f32, tag="bq")
        bk_t = consts.tile([P, DG // P], f32, tag="bk")
        bvr_t = consts.tile([P, HG, DK], f32, tag="bvr")
        warm_in = consts.tile([1, 4], f32, tag="warmin")
        warm = consts.tile([1, 4], f32, tag="warm")

        # PSUM pools: 4 (scores) + 2 (attnV accum) + 1 (proj) + 1 (yproj)
        sc_ps = ctx.enter_context(tc.tile_pool(name="scps", bufs=SC_BUFS,
                                               space="PSUM"))
        oa_ps = ctx.enter_context(tc.tile_pool(name="oaps", bufs=OA_BUFS,
                                               space="PSUM"))
        pp_ps = ctx.enter_context(tc.tile_pool(name="ppps", bufs=1, space="PSUM"))
        yp_ps = (ctx.enter_context(tc.tile_pool(name="ypps", bufs=1,
                                                space="PSUM"))
                 if SEP_YP else None)

        # prefetch the exp table-set before any real work
        nc.gpsimd.memset(warm_in[:], 0.0)
        nc.scalar.activation(warm[:], warm_in[:], Exp)
        # ones column of V (softmax denominator trick)
        nc.gpsimd.memset(Vg[:, :, :, DK], 1.0)

        def load_xin(src, qb, splits=1):
            xt = xin_pool.tile([P, D // P, 512], bf16, tag="xin")
            step = (D // P) // splits
            for s in range(splits):
                r0, r1 = s * step * P, (s + 1) * step * P
                nc.sync.dma_start(
                    xt[:, s * step:(s + 1) * step, :],
                    src[r0:r1, qb * 512:(qb + 1) * 512]
                    .rearrange("(o p) m -> p o m", p=P))
            return xt

        # alternate proj PSUM banks to avoid WAR chains. With a separate
        # yproj bank, alternate pp/yp everywhere; otherwise alternate with
        # sc banks only while attention is not running (prologue).
        alt_state = [0]
        in_prologue = [True]

        def proj_ps_tile():
            alt_state[0] ^= 1
            if SEP_YP:
                pool, tag = ((pp_ps, "pp"), (yp_ps, "yp"))[alt_state[0]]
            elif in_prologue[0] and alt_state[0]:
                pool, tag = sc_ps, "sc"
            else:
                pool, tag = pp_ps, "pp"
            return pool.tile([P, 512], f32, tag=tag, name=tag)

        def proj_qk_m(w_t, xt, bias_t, dst_tile, m):
            ps = proj_ps_tile()
            for j in range(D // P):
                nc.tensor.matmul(
                    ps[:], w_t[:, j, m * P:(m + 1) * P], xt[:, j, :],
                    start=(j == 0), stop=(j == D // P - 1))
            nc.vector.tensor_scalar_add(
                dst_tile[:], ps[:], bias_t[:, m:m + 1])

        def proj_v_part(xt, qb, mt_l, h2):
            kt = qb * 4 + mt_l
            ps = proj_ps_tile()
            psv = ps[:, 0:DG // 2]
            for j in range(D // P):
                nc.tensor.matmul(
                    psv, xt[:, j, mt_l * P:(mt_l + 1) * P],
                    wv_t[:, j, h2 * 256:(h2 + 1) * 256],
                    start=(j == 0), stop=(j == D // P - 1))
            nc.vector.tensor_add(
                Vg[:, kt, h2 * 4:(h2 + 1) * 4, 0:DK],
                psv.rearrange("p (h d) -> p h d", h=4),
                bvr_t[:, h2 * 4:(h2 + 1) * 4, :])

        ycopy_state = [0]

        def yproj_part(qb, OT, qt_l, nb, ps_pool, ps_tag, split_t=False,
                       tail_ys=None):
            """Emit output-projection part. If split_t, t=0..2 are emitted by
            the caller via the returned closure pair (prefill, finish)."""
            ps = ps_pool.tile([P, 512], f32, tag=ps_tag, name="yp")

            def mm(t):
                nc.tensor.matmul(
                    ps[:], OT[:, t, qt_l * P:(qt_l + 1) * P],
                    wo_t[:, t, nb * 512:(nb + 1) * 512],
                    start=(t == 0), stop=(t == DG // P - 1))

            def finish():
                mm(DG // P - 1)
                if tail_ys is not None:
                    # tail: alternate DVE / ACT so copies pipeline 2-wide
                    # (gpsimd cannot read PSUM)
                    ys_t, idx = tail_ys
                    ycopy_state[0] ^= 1
                    if ycopy_state[0]:
                        nc.vector.tensor_copy(ys_t[:, idx, :], ps[:])
                    else:
                        nc.scalar.activation(
                            ys_t[:, idx, :], ps[:],
                            mybir.ActivationFunctionType.Copy)
                else:
                    ys = y_pool.tile([P, 512], bf16, tag="ys", name="ys")
                    nc.vector.tensor_copy(ys[:], ps[:])
                    nc.sync.dma_start(
                        y[(qb * 4 + qt_l) * P:(qb * 4 + qt_l + 1) * P,
                          nb * 512:(nb + 1) * 512],
                        ys[:])

            def prefill():
                for t in range(DG // P - 1):
                    mm(t)

            if split_t:
                return prefill, finish
            prefill()
            finish()

        def attention_qb(qb, QTcur, fillers, late_fillers=()):
            """fillers: evenly woven closures. late_fillers: (deadline_unit,
            closure) — emitted once done-count reaches deadline (deadlines
            must be achievable: closure emitted before its consumer unit)."""
            nfill0 = len(fillers)
            kmax = 4 * (qb + 1)
            units = [(p, kt, hh)
                     for p in range(HG // 2) for kt in range(kmax)
                     for hh in (0, 1)]
            nunits = len(units)
            done = [0]
            late = list(late_fillers)
            OT = ot_pool.tile([P, HG // 2, 512], bf16, tag="OT", name="OT")
            oas = {}
            tps = {}
            ex_by = {}

            def emit_scores(p, kt, hh):
                diag = kt >= 4 * qb
                f0 = (kt - 4 * qb) * P if diag else 0
                sc = sc_ps.tile([P, 512], f32, tag="sc", name="sc")
                nc.tensor.matmul(
                    sc[:, f0:512],
                    KTs[p][kt // 4][hh * DK:(hh + 1) * DK,
                                    (kt % 4) * P:(kt % 4 + 1) * P],
                    QTcur[p][hh * DK:(hh + 1) * DK, f0:512],
                    start=True, stop=True, tile_position=(hh * DK, 0))
                ex = exp_pool.tile([P, 512], bf16, tag="ex", name="ex")
                nc.scalar.activation(ex[:, f0:512], sc[:, f0:512], Exp)
                if diag:
                    # causal mask, multiplicative post-exp: only the leading
                    # 128-col strip of the valid range is triangular
                    nc.vector.tensor_mul(
                        ex[:, f0:f0 + P], ex[:, f0:f0 + P], mask_t[:, 0:P])
                ex_by[(p, kt, hh)] = ex

            def emit_attnv(p, kt, hh):
                # operand-swapped: attn block [128kt x 128q] is the stationary
                # operand, V [128kt, 65] streams -> only 65 cols per matmul.
                # Output is q-major [128q, 65]; col 64 = softmax denominator.
                if kt == 0:
                    oas[(p, hh)] = oa_ps.tile([P, 4, DK + 1], f32,
                                              tag=f"oa{hh}", name=f"oa{hh}")
                    if hh == 0:
                        tps[p] = None
                oa = oas[(p, hh)]
                ex = ex_by.pop((p, kt, hh))
                # start=True pends-to-zero the WHOLE 2KB bank, so only the
                # first matmul of the bank may carry it: the other s regions
                # initialize via the pending-zero overwrite semantics.
                for s_q in range(4):
                    if kt <= 4 * qb + s_q:
                        nc.tensor.matmul(
                            oa[:, s_q, :], ex[:, s_q * P:(s_q + 1) * P],
                            Vg[:, kt, 2 * p + hh, :],
                            start=(kt == 0 and s_q == 0),
                            stop=(kt == kmax - 1))
                nf = 0
                if kt == kmax - 1:
                    rs2 = sb_small.tile([P, 4], f32, tag="rs", name="rs")
                    nc.vector.reciprocal(rs2[:], oa[:, :, DK])
                    outn = outn_pool.tile([P, 4, DK], bf16, tag=f"on{hh}",
                                          name="outn")
                    for s_q in range(4):
                        nc.vector.tensor_scalar_mul(
                            outn[:, s_q, :], oa[:, s_q, 0:DK],
                            rs2[:, s_q:s_q + 1])
                    # transpose back to chan-major into a borrowed yp bank
                    if hh == 0:
                        tps[p] = yp_ps.tile([P, 4, P], bf16, tag="yp",
                                            name="tp")
                    tp = tps[p]
                    for s_q in range(4):
                        nc.tensor.transpose(
                            tp[hh * DK:(hh + 1) * DK, s_q, :],
                            outn[:, s_q, :], ident_t[:])
                    if hh == 1:
                        nc.vector.tensor_copy(
                            OT[:, p, :],
                            tp[:].rearrange("c s q -> c (s q)"))
                    if hh == 0 and p + 1 < HG // 2:
                        # pair boundary: next pair's attnV kt=0 must wait for
                        # this pair's oa reads; pull extra filler cover here
                        nf = 2
                done[0] += 1
                while late and late[0][0] <= done[0]:
                    late.pop(0)[1]()
                target_done = (nfill0 * done[0]) // nunits
                while fillers and (nfill0 - len(fillers) < target_done + nf):
                    fillers.pop(0)()
                    if nf:
                        nf -= 1

            dd = min(PIPE_DEPTH, nunits)
            for j in range(dd):
                emit_scores(*units[j])
            while late and late[0][0] <= 0:
                late.pop(0)[1]()
            for i in range(nunits):
                if i + dd < nunits:
                    emit_scores(*units[i + dd])
                emit_attnv(*units[i])
            while late:
                late.pop(0)[1]()
            for f in fillers:
                f()
            del fillers[:]
            return OT

        def proj_block(qb):
            """Emit projections for block qb. Returns (QTcur, fillers,
            late_fillers): late ones run inside attention_qb(qb) itself,
            before their consumer pairs start."""
            fillers = []
            xtv = load_xin(xv, qb)
            for mt_l in range(4):
                for h2 in range(2):
                    fillers.append(
                        lambda mt_l=mt_l, h2=h2, xtv=xtv, qb=qb:
                        proj_v_part(xtv, qb, mt_l, h2))
            QTcur = []
            for m in range(DG // P):
                qt_t = qt_pool.tile([P, 512], bf16, tag=f"QTm{m}",
                                    name=f"QTm{m}")
                QTcur.append(qt_t)
            xtq = load_xin(xq, qb)
            for m in range(2):
                fillers.append(
                    lambda m=m, xtq=xtq: proj_qk_m(
                        wq_t, xtq, bq_t, QTcur[m], m))
            xtk = load_xin(xk, qb)
            for m in range(2):
                fillers.append(
                    lambda m=m, xtk=xtk, qb=qb:
                    proj_qk_m(wk_t, xtk, bk_t, KTs[m][qb], m))
            # pairs 2 and 3 of block qb are consumed late inside
            # attention_qb(qb): project them there (deadline = unit index
            # safely before first consumer emission p*2*kmax - DEPTH)
            kmax_n = 4 * (qb + 1)
            late = []
            for m in (2, 3):
                dl = max(1, m * 2 * kmax_n - PIPE_DEPTH - 4)
                late.append((dl - 8, lambda m=m, xtq=xtq: proj_qk_m(
                    wq_t, xtq, bq_t, QTcur[m], m)))
                late.append((dl, lambda m=m, xtk=xtk, qb=qb: proj_qk_m(
                    wk_t, xtk, bk_t, KTs[m][qb], m)))
            late.sort(key=lambda t: t[0])
            return QTcur, fillers, late

        # ---- block 0 prologue: interleave loads with their first users ----
        nc.sync.dma_start(wk_t[:, :, 0:256],
                          wk[:, 0:256].rearrange("(o p) m -> p o m", p=P))
        xtk0 = load_xin(xk, 0, splits=4)
        nc.sync.dma_start(bk_t[:], bk)
        nc.sync.dma_start(wk_t[:, :, 256:512],
                          wk[:, 256:512].rearrange("(o p) m -> p o m", p=P))
        for m in range(2):
            proj_qk_m(wk_t, xtk0, bk_t, KTs[m][0], m)
        xtv0 = load_xin(xv, 0, splits=2)
        nc.sync.dma_start(wv_t[:, :, 0:256],
                          wv[:, 0:256].rearrange("(o p) m -> p o m", p=P))
        nc.sync.dma_start(wv_t[:, :, 256:512],
                          wv[:, 256:512].rearrange("(o p) m -> p o m", p=P))
        nc.sync.dma_start(bvr_t[:], bvr)
        for mt_l in range(4):
            for h2 in range(2):
                proj_v_part(xtv0, 0, mt_l, h2)
        xtq0 = load_xin(xq, 0, splits=2)
        nc.sync.dma_start(wq_t[:, :, 0:256],
                          wq[:, 0:256].rearrange("(o p) m -> p o m", p=P))
        nc.sync.dma_start(wq_t[:, :, 256:512],
                          wq[:, 256:512].rearrange("(o p) m -> p o m", p=P))
        nc.sync.dma_start(bq_t[:], bq)
        nc.sync.dma_start(mask_t[:], masktri)
        nc.sync.dma_start(ident_t[:], ident)
        QTcur = []
        for m in range(DG // P):
            qt_t = qt_pool.tile([P, 512], bf16, tag=f"QTm{m}", name=f"QTm{m}")
            QTcur.append(qt_t)
        for m in range(2):
            proj_qk_m(wq_t, xtq0, bq_t, QTcur[m], m)
        nc.sync.dma_start(wo_t[:], wo.rearrange("(o p) m -> p o m", p=P))
        # pairs 2,3 of block 0 are projected inside attention_qb(0) itself
        late0 = []
        for m in (2, 3):
            dl = max(1, m * 2 * 4 - PIPE_DEPTH - 4)
            late0.append((max(1, dl - 4),
                          lambda m=m: proj_qk_m(wq_t, xtq0, bq_t, QTcur[m], m)))
            late0.append((dl,
                          lambda m=m: proj_qk_m(wk_t, xtk0, bk_t, KTs[m][0], m)))
        late0.sort(key=lambda t: t[0])

        def yproj_filler(q, O, part_i):
            qt_l, nb = part_i // 2, part_i % 2
            if SEP_YP:
                pool, tag = ((pp_ps, "pp"), (yp_ps, "yp"))[part_i % 2]
            else:
                pool, tag = pp_ps, "pp"
            return (lambda qt_l=qt_l, nb=nb, O=O, q=q, pool=pool, tag=tag:
                    yproj_part(q, O, qt_l, nb, pool, tag))

        in_prologue[0] = False
        prevOT = None     # OT of qb-1
        prev2OT = None    # OT of qb-2 (second half of its yproj deferred)
        pending_late = late0
        for qb in range(NQB):
            last = qb == NQB - 1
            fillers = []
            if prev2OT is not None:
                for part_i in range(2, 8):
                    fillers.append(yproj_filler(qb - 2, prev2OT, part_i))
            if prevOT is not None:
                nparts = 8 if last else 2
                for part_i in range(nparts):
                    fillers.append(yproj_filler(qb - 1, prevOT, part_i))
            nextQT = None
            late = ()
            if not last:
                nextQT, pf, late = proj_block(qb + 1)
                fillers.extend(pf)
            OT = attention_qb(qb, QTcur, fillers, late_fillers=pending_late)
            pending_late = late
            prev2OT, prevOT = prevOT, OT
            QTcur = nextQT

        # ---- tail: output projection of the last block, t-split across 4
        # PSUM banks (yp, pp and two sc rotations are all free now);
        # y stores grouped into two wide DMAs ----
        ys_tail = persist.tile([P, 8, 512], bf16, tag="ystail", name="ystail")
        tailpools = ([(yp_ps, "yp"), (pp_ps, "pp"), (sc_ps, "sc"),
                      (sc_ps, "sc")] if SEP_YP else
                     [(pp_ps, "pp"), (sc_ps, "sc"), (sc_ps, "sc"),
                      (sc_ps, "sc")])
        parts = [(qt_l, nb) for qt_l in range(4) for nb in range(2)]
        pf_fin = []
        for i, (qt_l, nb) in enumerate(parts):
            pool, tag = tailpools[i % 4]
            pf, fin = yproj_part(NQB - 1, prevOT, qt_l, nb, pool, tag,
                                 split_t=True, tail_ys=(ys_tail, i))
            pf_fin.append((pf, fin))
        # prefill t=0..2 of the first 4 parts (only needs pairs 0..2 of OT),
        # then stream finishes; later parts prefill as their bank frees
        for i in range(4):
            pf_fin[i][0]()
        for i in range(len(parts)):
            pf_fin[i][1]()
            if i + 4 < len(parts):
                pf_fin[i + 4][0]()
            if i == 3:
                nc.sync.dma_start(
                    y[(NQB - 1) * 512:(NQB - 1) * 512 + 256, :]
                    .rearrange("(q p) (n m) -> p q n m", p=P, n=2),
                    ys_tail[:, 0:4, :]
                    .rearrange("p (q n) m -> p q n m", q=2))
            if i == 5:
                nc.sync.dma_start(
                    y[(NQB - 1) * 512 + 256:(NQB - 1) * 512 + 384, :]
                    .rearrange("p (n m) -> p n m", n=2),
                    ys_tail[:, 4:6, :])
            if i == 7:
                nc.sync.dma_start(
                    y[(NQB - 1) * 512 + 384:NQB * 512, :]
                    .rearrange("p (n m) -> p n m", n=2),
                    ys_tail[:, 6:8, :])

    nc.compile()
    return nc


def _prep_inputs(query, key, value, Wq, bq, Wk, bk, Wv, bv, Wo, bo):
    scale = 1.0 / np.sqrt(DK)
    qr = _to_bf16(np.asarray(query))
    kr = _to_bf16(np.asarray(key))
    vr = _to_bf16(np.asarray(value))
    wq_full = _to_bf16(np.asarray(Wq).T * scale)   # [D, D], cols = out chans
    wk_full = _to_bf16(np.asarray(Wk).T)
    wv_full = _to_bf16(np.asarray(Wv).T)
    wo_full = _to_bf16(np.asarray(Wo).T)           # [Din, Dout]
    bq_s = np.asarray(bq) * scale

    jj = np.arange(P)[:, None]
    ff = np.arange(P)[None, :]
    masktri = _to_bf16(np.where(jj <= ff, 1.0, 0.0))
    ident = _to_bf16(np.eye(P, dtype=np.float32))

    in_maps = []
    for core in range(8):
        b, hg = core // 2, core % 2
        sl = slice(hg * DG, (hg + 1) * DG)
        in_maps.append({
            "xq": np.ascontiguousarray(qr[b].T),
            "xk": np.ascontiguousarray(kr[b].T),
            "xv": np.ascontiguousarray(vr[b].T),
            "wq": np.ascontiguousarray(wq_full[:, sl]),
            "wk": np.ascontiguousarray(wk_full[:, sl]),
            "wv": np.ascontiguousarray(wv_full[:, sl]),
            "wo": np.ascontiguousarray(wo_full[sl, :]),
            "bq": np.ascontiguousarray(
                bq_s[sl].reshape(DG // P, P).T.astype(np.float32)),
            "bk": np.ascontiguousarray(
                np.asarray(bk)[sl].reshape(DG // P, P).T.astype(np.float32)),
            "bvr": np.broadcast_to(
                np.asarray(bv)[sl].astype(np.float32).reshape(HG, DK),
                (P, HG, DK)).copy(),
            "masktri": masktri,
            "ident": ident,
        })
    return in_maps


def kernel(query, key, value, mask, Wq, bq, Wk, bk, Wv, bv, Wo, bo,
           **run_kwargs):
    from concourse.bass_utils import run_bass_kernel_spmd

    if "nc" not in _cache:
        _cache["nc"] = _build()
    nc = _cache["nc"]

    in_maps = _prep_inputs(query, key, value, Wq, bq, Wk, bk, Wv, bv, Wo, bo)
    res = run_bass_kernel_spmd(nc, in_maps, core_ids=list(range(8)),
                               **run_kwargs)
    bo = np.asarray(bo, dtype=np.float32)
    out = np.empty((B, S, D), dtype=np.float32)
    for b in range(B):
        out[b] = (res.results[2 * b]["y"].astype(np.float32)
                  + res.results[2 * b + 1]["y"].astype(np.float32) + bo)
    _cache["last_results"] = res
    return out


# revision 21
# speedup vs baseline: 1.3802x; 1.0737x over previous
"""Multi-head causal attention (B=4, S=2048, D=1024, H=16) on 8 TRN2 cores.

Sharding: core = (batch, head-group): 4 batches x 2 groups of 8 heads.
Every core runs an identical program (uniform causal structure -> valid SPMD):
  - Q/K/V projections for its 8 heads over its batch's full 2048 rows
    (Q,K produced transposed [Dout, S]; V natural [S, Dout] + ones column)
  - causal attention per (pair, keytile, head) unit: scores_T = K_h @ Q_h^T,
    ACT exp straight from PSUM into bf16; multiplicative 0/1 triangle mask on
    the diagonal 128-strip post-exp (DVE); attnV with M=65 (65th row
    accumulates the softmax denominator); reciprocal + gpsimd
    partition-broadcast; normalize into OT. Units are software-pipelined
    (scores i+DEPTH issued before attnV i) with projection/output-projection
    work woven in as PE filler under deadline constraints.
  - partial output projection y_part = OT^T @ Wo^T (contraction over this
    group's 512 channels)
Host: y[b] = y_part[b,g0] + y_part[b,g1] + bo.

All matmuls run in bf16 (inputs/weights rounded on host; intermediates
rounded by the producing DVE/ACT op); PSUM accumulation is fp32.
"""
import sys

for _p in ("/opt/trn_rl_repo", "/root/.axon_site/_ro/trn_rl_repo"):
    if _p not in sys.path:
        sys.path.append(_p)

import numpy as np
from contextlib import ExitStack

B, S, D, H = 4, 2048, 1024, 16
DK = D // H          # 64
HG = H // 2          # 8 heads per core
DG = HG * DK         # 512 channels per core
P = 128
NQB = S // 512       # 4 query blocks of 512
NKT = S // P         # 16 key tiles of 128

# PSUM-bank/pipeline configuration (8 banks total):
#   sc = SC_BUFS, oa = 2*OA_BUFS, proj = 1, yproj = SEP_YP
import os
PIPE_DEPTH = int(os.environ.get("K_DEPTH", "2"))   # in 2-bank score BLOCKS
OA_BUFS = int(os.environ.get("K_OA", "1"))
SEP_YP = int(os.environ.get("K_SEPYP", "1"))
SC_BUFS = (8 - 2 * OA_BUFS - 1 - SEP_YP) // 2

_cache = {}


def _to_bf16(x):
    import ml_dtypes
    return np.ascontiguousarray(np.asarray(x, dtype=np.float32)).astype(
        ml_dtypes.bfloat16)


def _build():
    import concourse.tile as tile
    from concourse import bacc, mybir

    f32 = mybir.dt.float32
    bf16 = mybir.dt.bfloat16
    Exp = mybir.ActivationFunctionType.Exp

    nc = bacc.Bacc("TRN2", target_bir_lowering=False, debug=False,
                   enable_asserts=False, num_devices=8)

    xq = nc.dram_tensor("xq", [D, S], bf16, kind="ExternalInput").ap()
    xk = nc.dram_tensor("xk", [D, S], bf16, kind="ExternalInput").ap()
    xv = nc.dram_tensor("xv", [D, S], bf16, kind="ExternalInput").ap()
    wq = nc.dram_tensor("wq", [D, DG], bf16, kind="ExternalInput").ap()
    wk = nc.dram_tensor("wk", [D, DG], bf16, kind="ExternalInput").ap()
    wv = nc.dram_tensor("wv", [D, DG], bf16, kind="ExternalInput").ap()
    wo = nc.dram_tensor("wo", [DG, D], bf16, kind="ExternalInput").ap()
    bq = nc.dram_tensor("bq", [P, DG // P], f32, kind="ExternalInput").ap()
    bk = nc.dram_tensor("bk", [P, DG // P], f32, kind="ExternalInput").ap()
    bvr = nc.dram_tensor("bvr", [P, HG, DK], f32, kind="ExternalInput").ap()
    masktri = nc.dram_tensor("masktri", [P, P], bf16, kind="ExternalInput").ap()
    ident = nc.dram_tensor("ident", [P, P], bf16, kind="ExternalInput").ap()
    y = nc.dram_tensor("y", [S, D], bf16, kind="ExternalOutput").ap()

    with tile.TileContext(nc) as tc, ExitStack() as ctx:
        persist = ctx.enter_context(tc.tile_pool(name="persist", bufs=1))
        consts = ctx.enter_context(tc.tile_pool(name="consts", bufs=1))
        xin_pool = ctx.enter_context(tc.tile_pool(name="xin", bufs=6))
        qt_pool = ctx.enter_context(tc.tile_pool(name="qtp", bufs=2))
        ot_pool = ctx.enter_context(tc.tile_pool(name="otpool", bufs=3))
        sb_small = ctx.enter_context(tc.tile_pool(name="sbs", bufs=2))
        outn_pool = ctx.enter_context(tc.tile_pool(name="outn", bufs=2))
        y_pool = ctx.enter_context(tc.tile_pool(name="ysb", bufs=3))
        exp_pool = ctx.enter_context(tc.tile_pool(name="expp", bufs=6))

        # persistent weights / K / V
        wq_t = persist.tile([P, D // P, DG], bf16, tag="wq", name="wq")
        wk_t = persist.tile([P, D // P, DG], bf16, tag="wk", name="wk")
        wv_t = persist.tile([P, D // P, DG], bf16, tag="wv", name="wv")
        wo_t = persist.tile([P, DG // P, D], bf16, tag="wo", name="wo")
        KTs = [[persist.tile([P, 512], bf16, tag=f"KT{m}_{qc}",
                             name=f"KT{m}_{qc}")
                for qc in range(4)] for m in range(4)]
        Vg = persist.tile([P, NKT, HG, DK + 1], bf16, tag="Vg", name="Vg")

        mask_t = consts.tile([P, 2, P], bf16, tag="mask")
        ident_t = consts.tile([P, P], bf16, tag="ident")
        bq_t = consts.tile([P, DG // P], f32, tag="bq")
        bk_t = consts.tile([P, DG // P], f32, tag="bk")
        bvr_t = consts.tile([P, HG, DK], f32, tag="bvr")
        warm_in = consts.tile([1, 4], f32, tag="warmin")
        warm = consts.tile([1, 4], f32, tag="warm")

        # PSUM pools: 4 (scores) + 2 (attnV accum) + 1 (proj) + 1 (yproj)
        sc_ps = ctx.enter_context(tc.tile_pool(name="scps", bufs=SC_BUFS,
                                               space="PSUM"))
        oa_ps = ctx.enter_context(tc.tile_pool(name="oaps", bufs=OA_BUFS,
                                               space="PSUM"))
        pp_ps = ctx.enter_context(tc.tile_pool(name="ppps", bufs=1, space="PSUM"))
        yp_ps = (ctx.enter_context(tc.tile_pool(name="ypps", bufs=1,
                                                space="PSUM"))
                 if SEP_YP else None)

        # prefetch the exp table-set before any real work
        nc.gpsimd.memset(warm_in[:], 0.0)
        nc.scalar.activation(warm[:], warm_in[:], Exp)
        # ones column of V (softmax denominator trick)
        nc.gpsimd.memset(Vg[:, :, :, DK], 1.0)

        def load_xin(src, qb, splits=1):
            xt = xin_pool.tile([P, D // P, 512], bf16, tag="xin")
            step = (D // P) // splits
            for s in range(splits):
                r0, r1 = s * step * P, (s + 1) * step * P
                nc.sync.dma_start(
                    xt[:, s * step:(s + 1) * step, :],
                    src[r0:r1, qb * 512:(qb + 1) * 512]
                    .rearrange("(o p) m -> p o m", p=P))
            return xt

        # alternate proj PSUM banks to avoid WAR chains. With a separate
        # yproj bank, alternate pp/yp everywhere; otherwise alternate with
        # sc banks only while attention is not running (prologue).
        alt_state = [0]
        in_prologue = [True]

        def proj_ps_tile():
            alt_state[0] ^= 1
            if SEP_YP:
                pool, tag = ((pp_ps, "pp"), (yp_ps, "yp"))[alt_state[0]]
            elif in_prologue[0] and alt_state[0]:
                pool, tag = sc_ps, "sc"
            else:
                pool, tag = pp_ps, "pp"
            return pool.tile([P, 512], f32, tag=tag, name=tag)

        def proj_qk_m(w_t, xt, bias_t, dst_tile, m):
            ps = proj_ps_tile()
            for j in range(D // P):
                nc.tensor.matmul(
                    ps[:], w_t[:, j, m * P:(m + 1) * P], xt[:, j, :],
                    start=(j == 0), stop=(j == D // P - 1))
            nc.vector.tensor_scalar_add(
                dst_tile[:], ps[:], bias_t[:, m:m + 1])

        def proj_v_part(xt, qb, mt_l, h2):
            kt = qb * 4 + mt_l
            ps = proj_ps_tile()
            psv = ps[:, 0:DG // 2]
            for j in range(D // P):
                nc.tensor.matmul(
                    psv, xt[:, j, mt_l * P:(mt_l + 1) * P],
                    wv_t[:, j, h2 * 256:(h2 + 1) * 256],
                    start=(j == 0), stop=(j == D // P - 1))
            nc.vector.tensor_add(
                Vg[:, kt, h2 * 4:(h2 + 1) * 4, 0:DK],
                psv.rearrange("p (h d) -> p h d", h=4),
                bvr_t[:, h2 * 4:(h2 + 1) * 4, :])

        ycopy_state = [0]

        def yproj_part(qb, OT, qt_l, nb, ps_pool, ps_tag, split_t=False,
                       tail_ys=None):
            """Emit output-projection part. If split_t, t=0..2 are emitted by
            the caller via the returned closure pair (prefill, finish)."""
            ps = ps_pool.tile([P, 512], f32, tag=ps_tag, name="yp")

            def mm(t):
                nc.tensor.matmul(
                    ps[:], OT[:, t, qt_l * P:(qt_l + 1) * P],
                    wo_t[:, t, nb * 512:(nb + 1) * 512],
                    start=(t == 0), stop=(t == DG // P - 1))

            def finish():
                mm(DG // P - 1)
                if tail_ys is not None:
                    # tail: alternate DVE / ACT so copies pipeline 2-wide
                    # (gpsimd cannot read PSUM)
                    ys_t, idx = tail_ys
                    ycopy_state[0] ^= 1
                    if ycopy_state[0]:
                        nc.vector.tensor_copy(ys_t[:, idx, :], ps[:])
                    else:
                        nc.scalar.activation(
                            ys_t[:, idx, :], ps[:],
                            mybir.ActivationFunctionType.Copy)
                else:
                    ys = y_pool.tile([P, 512], bf16, tag="ys", name="ys")
                    nc.vector.tensor_copy(ys[:], ps[:])
                    nc.sync.dma_start(
                        y[(qb * 4 + qt_l) * P:(qb * 4 + qt_l + 1) * P,
                          nb * 512:(nb + 1) * 512],
                        ys[:])

            def prefill():
                for t in range(DG // P - 1):
                    mm(t)

            if split_t:
                return prefill, finish
            prefill()
            finish()

        def attention_qb(qb, QTcur, fillers, late_fillers=()):
            """fillers: evenly woven closures. late_fillers: (deadline_unit,
            closure) — emitted once done-count reaches deadline (deadlines
            must be achievable: closure emitted before its consumer unit)."""
            nfill0 = len(fillers)
            kmax = 4 * (qb + 1)
            units = [(p, kt, hh)
                     for p in range(HG // 2) for kt in range(kmax)
                     for hh in (0, 1)]
            nunits = len(units)
            done = [0]
            late = list(late_fillers)
            OT = ot_pool.tile([P, HG // 2, 512], bf16, tag="OT", name="OT")
            oas = {}
            tps = {}
            ex_by = {}

            def emit_scores(p, kt, hh):
                diag = kt >= 4 * qb
                f0 = (kt - 4 * qb) * P if diag else 0
                sc = sc_ps.tile([P, 512], f32, tag="sc", name="sc")
                nc.tensor.matmul(
                    sc[:, f0:512],
                    KTs[p][kt // 4][hh * DK:(hh + 1) * DK,
                                    (kt % 4) * P:(kt % 4 + 1) * P],
                    QTcur[p][hh * DK:(hh + 1) * DK, f0:512],
                    start=True, stop=True, tile_position=(hh * DK, 0))
                ex = exp_pool.tile([P, 512], bf16, tag="ex", name="ex")
                nc.scalar.activation(ex[:, f0:512], sc[:, f0:512], Exp)
                if diag:
                    # causal mask, multiplicative post-exp: only the leading
                    # 128-col strip of the valid range is triangular
                    nc.vector.tensor_mul(
                        ex[:, f0:f0 + P], ex[:, f0:f0 + P], mask_t[:, 0:P])
                ex_by[(p, kt, hh)] = ex

            def emit_attnv(p, kt, hh):
                # operand-swapped: attn block [128kt x 128q] is the stationary
                # operand, V [128kt, 65] streams -> only 65 cols per matmul.
                # Output is q-major [128q, 65]; col 64 = softmax denominator.
                if kt == 0:
                    oas[(p, hh)] = oa_ps.tile([P, 4, DK + 1], f32,
                                              tag=f"oa{hh}", name=f"oa{hh}")
                    if hh == 0:
                        tps[p] = None
                oa = oas[(p, hh)]
                ex = ex_by.pop((p, kt, hh))
                # start=True pends-to-zero the WHOLE 2KB bank, so only the
                # first matmul of the bank may carry it: the other s regions
                # initialize via the pending-zero overwrite semantics.
                for s_q in range(4):
                    if kt <= 4 * qb + s_q:
                        nc.tensor.matmul(
                            oa[:, s_q, :], ex[:, s_q * P:(s_q + 1) * P],
                            Vg[:, kt, 2 * p + hh, :],
                            start=(kt == 0 and s_q == 0),
                            stop=(kt == kmax - 1))
                if kt == kmax - 1:
                    rs2 = sb_small.tile([P, 4], f32, tag="rs", name="rs")
                    nc.vector.reciprocal(rs2[:], oa[:, :, DK])
                    outn = outn_pool.tile([P, 4, DK], bf16, tag=f"on{hh}",
                                          name="outn")
                    for s_q in range(4):
                        nc.vector.tensor_scalar_mul(
                            outn[:, s_q, :], oa[:, s_q, 0:DK],
                            rs2[:, s_q:s_q + 1])
                    # transpose back to chan-major into a borrowed yp bank
                    if hh == 0:
                        tps[p] = yp_ps.tile([P, 4, P], bf16, tag="yp",
                                            name="tp")
                    tp = tps[p]
                    for s_q in range(4):
                        nc.tensor.transpose(
                            tp[hh * DK:(hh + 1) * DK, s_q, :],
                            outn[:, s_q, :], ident_t[:])
                    if hh == 1:
                        nc.vector.tensor_copy(
                            OT[:, p, :],
                            tp[:].rearrange("c s q -> c (s q)"))


            def pop_work(i):
                # emit filler work BEFORE attnV(i): it sits between the
                # lookahead scores and the exp-dependent attnV, covering
                # the exp latency in-position
                done[0] = i + 1
                p, kt, hh = units[i]
                nf = 2 if (kt == kmax - 1 and hh == 0
                           and p + 1 < HG // 2) else 0
                while late and late[0][0] <= done[0]:
                    late.pop(0)[1]()
                target_done = (nfill0 * done[0]) // nunits
                while fillers and (nfill0 - len(fillers) < target_done + nf):
                    fillers.pop(0)()
                    if nf:
                        nf -= 1

            dd = min(PIPE_DEPTH, nunits)
            for j in range(dd):
                emit_scores(*units[j])
            while late and late[0][0] <= 0:
                late.pop(0)[1]()
            for i in range(nunits):
                if i + dd < nunits:
                    emit_scores(*units[i + dd])
                pop_work(i)
                emit_attnv(*units[i])
            while late:
                late.pop(0)[1]()
            for f in fillers:
                f()
            del fillers[:]
            return OT

        def proj_block(qb):
            """Emit projections for block qb. Returns (QTcur, fillers,
            late_fillers): late ones run inside attention_qb(qb) itself,
            before their consumer pairs start."""
            fillers = []
            xtv = load_xin(xv, qb)
            for mt_l in range(4):
                for h2 in range(2):
                    fillers.append(
                        lambda mt_l=mt_l, h2=h2, xtv=xtv, qb=qb:
                        proj_v_part(xtv, qb, mt_l, h2))
            QTcur = []
            for m in range(DG // P):
                qt_t = qt_pool.tile([P, 512], bf16, tag=f"QTm{m}",
                                    name=f"QTm{m}")
                QTcur.append(qt_t)
            xtq = load_xin(xq, qb)
            for m in range(2):
                fillers.append(
                    lambda m=m, xtq=xtq: proj_qk_m(
                        wq_t, xtq, bq_t, QTcur[m], m))
            xtk = load_xin(xk, qb)
            for m in range(2):
                fillers.append(
                    lambda m=m, xtk=xtk, qb=qb:
                    proj_qk_m(wk_t, xtk, bk_t, KTs[m][qb], m))
            # pairs 2 and 3 of block qb are consumed late inside
            # attention_qb(qb): project them there (deadline = unit index
            # safely before first consumer emission p*2*kmax - DEPTH)
            kmax_n = 4 * (qb + 1)
            late = []
            for m in (2, 3):
                dl = max(1, m * kmax_n - PIPE_DEPTH - 2)
                late.append((max(1, dl - 4), lambda m=m, xtq=xtq: proj_qk_m(
                    wq_t, xtq, bq_t, QTcur[m], m)))
                late.append((dl, lambda m=m, xtk=xtk, qb=qb: proj_qk_m(
                    wk_t, xtk, bk_t, KTs[m][qb], m)))
            late.sort(key=lambda t: t[0])
            return QTcur, fillers, late

        # ---- block 0 prologue: interleave loads with their first users ----
        nc.sync.dma_start(wk_t[:, :, 0:256],
                          wk[:, 0:256].rearrange("(o p) m -> p o m", p=P))
        xtk0 = load_xin(xk, 0, splits=4)
        nc.sync.dma_start(bk_t[:], bk)
        nc.sync.dma_start(wk_t[:, :, 256:512],
                          wk[:, 256:512].rearrange("(o p) m -> p o m", p=P))
        for m in range(2):
            proj_qk_m(wk_t, xtk0, bk_t, KTs[m][0], m)
        xtv0 = load_xin(xv, 0, splits=2)
        nc.sync.dma_start(wv_t[:, :, 0:256],
                          wv[:, 0:256].rearrange("(o p) m -> p o m", p=P))
        nc.sync.dma_start(wv_t[:, :, 256:512],
                          wv[:, 256:512].rearrange("(o p) m -> p o m", p=P))
        nc.sync.dma_start(bvr_t[:], bvr)
        for mt_l in range(4):
            for h2 in range(2):
                proj_v_part(xtv0, 0, mt_l, h2)
        xtq0 = load_xin(xq, 0, splits=2)
        nc.sync.dma_start(wq_t[:, :, 0:256],
                          wq[:, 0:256].rearrange("(o p) m -> p o m", p=P))
        nc.sync.dma_start(wq_t[:, :, 256:512],
                          wq[:, 256:512].rearrange("(o p) m -> p o m", p=P))
        nc.sync.dma_start(bq_t[:], bq)
        nc.sync.dma_start(
            mask_t[:], masktri[:, None, :].to_broadcast((P, 2, P)))
        nc.sync.dma_start(ident_t[:], ident)
        QTcur = []
        for m in range(DG // P):
            qt_t = qt_pool.tile([P, 512], bf16, tag=f"QTm{m}", name=f"QTm{m}")
            QTcur.append(qt_t)
        for m in range(2):
            proj_qk_m(wq_t, xtq0, bq_t, QTcur[m], m)
        nc.sync.dma_start(wo_t[:], wo.rearrange("(o p) m -> p o m", p=P))
        # pairs 2,3 of block 0 are projected inside attention_qb(0) itself
        late0 = []
        for m in (2, 3):
            dl = max(1, m * 4 - PIPE_DEPTH - 2)
            late0.append((max(1, dl - 2),
                          lambda m=m: proj_qk_m(wq_t, xtq0, bq_t, QTcur[m], m)))
            late0.append((dl,
                          lambda m=m: proj_qk_m(wk_t, xtk0, bk_t, KTs[m][0], m)))
        late0.sort(key=lambda t: t[0])

        def yproj_filler(q, O, part_i):
            qt_l, nb = part_i // 2, part_i % 2
            if SEP_YP:
                pool, tag = ((pp_ps, "pp"), (yp_ps, "yp"))[part_i % 2]
            else:
                pool, tag = pp_ps, "pp"
            return (lambda qt_l=qt_l, nb=nb, O=O, q=q, pool=pool, tag=tag:
                    yproj_part(q, O, qt_l, nb, pool, tag))

        in_prologue[0] = False
        prevOT = None     # OT of qb-1
        prev2OT = None    # OT of qb-2 (second half of its yproj deferred)
        pending_late = late0
        for qb in range(NQB):
            last = qb == NQB - 1
            fillers = []
            if prev2OT is not None:
                for part_i in range(2, 8):
                    fillers.append(yproj_filler(qb - 2, prev2OT, part_i))
            if prevOT is not None:
                nparts = 8 if last else 2
                for part_i in range(nparts):
                    fillers.append(yproj_filler(qb - 1, prevOT, part_i))
            nextQT = None
            late = ()
            if not last:
                nextQT, pf, late = proj_block(qb + 1)
                fillers.extend(pf)
            OT = attention_qb(qb, QTcur, fillers, late_fillers=pending_late)
            pending_late = late
            prev2OT, prevOT = prevOT, OT
            QTcur = nextQT

        # ---- tail: output projection of the last block, t-split across 4
        # PSUM banks (yp, pp and two sc rotations are all free now);
        # y stores grouped into two wide DMAs ----
        ys_tail = persist.tile([P, 8, 512], bf16, tag="ystail", name="ystail")
        tailpools = ([(yp_ps, "yp"), (pp_ps, "pp"), (sc_ps, "sc"),
                      (sc_ps, "sc")] if SEP_YP else
                     [(pp_ps, "pp"), (sc_ps, "sc"), (sc_ps, "sc"),
                      (sc_ps, "sc")])
        parts = [(qt_l, nb) for qt_l in range(4) for nb in range(2)]
        pf_fin = []
        for i, (qt_l, nb) in enumerate(parts):
            pool, tag = tailpools[i % 4]
            pf, fin = yproj_part(NQB - 1, prevOT, qt_l, nb, pool, tag,
                                 split_t=True, tail_ys=(ys_tail, i))
            pf_fin.append((pf, fin))
        # prefill t=0..2 of the first 4 parts (only needs pairs 0..2 of OT),
        # then stream finishes; later parts prefill as their bank frees
        for i in range(4):
            pf_fin[i][0]()
        for i in range(len(parts)):
            pf_fin[i][1]()
            if i + 4 < len(parts):
                pf_fin[i + 4][0]()
            if i == 3:
                nc.sync.dma_start(
                    y[(NQB - 1) * 512:(NQB - 1) * 512 + 256, :]
                    .rearrange("(q p) (n m) -> p q n m", p=P, n=2),
                    ys_tail[:, 0:4, :]
                    .rearrange("p (q n) m -> p q n m", q=2))
            if i == 5:
                nc.sync.dma_start(
                    y[(NQB - 1) * 512 + 256:(NQB - 1) * 512 + 384, :]
                    .rearrange("p (n m) -> p n m", n=2),
                    ys_tail[:, 4:6, :])
            if i == 7:
                nc.sync.dma_start(
                    y[(NQB - 1) * 512 + 384:NQB * 512, :]
                    .rearrange("p (n m) -> p n m", n=2),
                    ys_tail[:, 6:8, :])

    nc.compile()
    return nc


def _prep_inputs(query, key, value, Wq, bq, Wk, bk, Wv, bv, Wo, bo):
    scale = 1.0 / np.sqrt(DK)
    qr = _to_bf16(np.asarray(query))
    kr = _to_bf16(np.asarray(key))
    vr = _to_bf16(np.asarray(value))
    wq_full = _to_bf16(np.asarray(Wq).T * scale)   # [D, D], cols = out chans
    wk_full = _to_bf16(np.asarray(Wk).T)
    wv_full = _to_bf16(np.asarray(Wv).T)
    wo_full = _to_bf16(np.asarray(Wo).T)           # [Din, Dout]
    bq_s = np.asarray(bq) * scale

    jj = np.arange(P)[:, None]
    ff = np.arange(P)[None, :]
    masktri = _to_bf16(np.where(jj <= ff, 1.0, 0.0))
    ident = _to_bf16(np.eye(P, dtype=np.float32))

    in_maps = []
    for core in range(8):
        b, hg = core // 2, core % 2
        sl = slice(hg * DG, (hg + 1) * DG)
        in_maps.append({
            "xq": np.ascontiguousarray(qr[b].T),
            "xk": np.ascontiguousarray(kr[b].T),
            "xv": np.ascontiguousarray(vr[b].T),
            "wq": np.ascontiguousarray(wq_full[:, sl]),
            "wk": np.ascontiguousarray(wk_full[:, sl]),
            "wv": np.ascontiguousarray(wv_full[:, sl]),
            "wo": np.ascontiguousarray(wo_full[sl, :]),
            "bq": np.ascontiguousarray(
                bq_s[sl].reshape(DG // P, P).T.astype(np.float32)),
            "bk": np.ascontiguousarray(
                np.asarray(bk)[sl].reshape(DG // P, P).T.astype(np.float32)),
            "bvr": np.broadcast_to(
                np.asarray(bv)[sl].astype(np.float32).reshape(HG, DK),
                (P, HG, DK)).copy(),
            "masktri": masktri,
            "ident": ident,
        })
    return in_maps


def kernel(query, key, value, mask, Wq, bq, Wk, bk, Wv, bv, Wo, bo,
           **run_kwargs):
    from concourse.bass_utils import run_bass_kernel_spmd

    if "nc" not in _cache:
        _cache["nc"] = _build()
    nc = _cache["nc"]

    in_maps = _prep_inputs(query, key, value, Wq, bq, Wk, bk, Wv, bv, Wo, bo)
    res = run_bass_kernel_spmd(nc, in_maps, core_ids=list(range(8)),
                               **run_kwargs)
    bo = np.asarray(bo, dtype=np.float32)
    out = np.empty((B, S, D), dtype=np.float32)
    for b in range(B):
        out[b] = (res.results[2 * b]["y"].astype(np.float32)
                  + res.results[2 * b + 1]["y"].astype(np.float32) + bo)
    _cache["last_results"] = res
    return out


# revision 22
# speedup vs baseline: 1.3847x; 1.0033x over previous
"""Multi-head causal attention (B=4, S=2048, D=1024, H=16) on 8 TRN2 cores.

Sharding: core = (batch, head-group): 4 batches x 2 groups of 8 heads.
Every core runs an identical program (uniform causal structure -> valid SPMD):
  - Q/K/V projections for its 8 heads over its batch's full 2048 rows
    (Q,K produced transposed [Dout, S]; V natural [S, Dout] + ones column)
  - causal attention per (pair, keytile, head) unit: scores_T = K_h @ Q_h^T,
    ACT exp straight from PSUM into bf16; multiplicative 0/1 triangle mask on
    the diagonal 128-strip post-exp (DVE); attnV with M=65 (65th row
    accumulates the softmax denominator); reciprocal + gpsimd
    partition-broadcast; normalize into OT. Units are software-pipelined
    (scores i+DEPTH issued before attnV i) with projection/output-projection
    work woven in as PE filler under deadline constraints.
  - partial output projection y_part = OT^T @ Wo^T (contraction over this
    group's 512 channels)
Host: y[b] = y_part[b,g0] + y_part[b,g1] + bo.

All matmuls run in bf16 (inputs/weights rounded on host; intermediates
rounded by the producing DVE/ACT op); PSUM accumulation is fp32.
"""
import sys

for _p in ("/opt/trn_rl_repo", "/root/.axon_site/_ro/trn_rl_repo"):
    if _p not in sys.path:
        sys.path.append(_p)

import numpy as np
from contextlib import ExitStack

B, S, D, H = 4, 2048, 1024, 16
DK = D // H          # 64
HG = H // 2          # 8 heads per core
DG = HG * DK         # 512 channels per core
P = 128
NQB = S // 512       # 4 query blocks of 512
NKT = S // P         # 16 key tiles of 128

# PSUM-bank/pipeline configuration (8 banks total):
#   sc = SC_BUFS, oa = 2*OA_BUFS, proj = 1, yproj = SEP_YP
import os
PIPE_DEPTH = int(os.environ.get("K_DEPTH", "2"))   # in 2-bank score BLOCKS
OA_BUFS = int(os.environ.get("K_OA", "1"))
SEP_YP = int(os.environ.get("K_SEPYP", "1"))
SC_BUFS = (8 - 2 * OA_BUFS - 1 - SEP_YP) // 2

_cache = {}


def _to_bf16(x):
    import ml_dtypes
    return np.ascontiguousarray(np.asarray(x, dtype=np.float32)).astype(
        ml_dtypes.bfloat16)


def _build():
    import concourse.tile as tile
    from concourse import bacc, mybir

    f32 = mybir.dt.float32
    bf16 = mybir.dt.bfloat16
    Exp = mybir.ActivationFunctionType.Exp

    nc = bacc.Bacc("TRN2", target_bir_lowering=False, debug=False,
                   enable_asserts=False, num_devices=8)

    xq = nc.dram_tensor("xq", [D, S], bf16, kind="ExternalInput").ap()
    xk = nc.dram_tensor("xk", [D, S], bf16, kind="ExternalInput").ap()
    xv = nc.dram_tensor("xv", [D, S], bf16, kind="ExternalInput").ap()
    wq = nc.dram_tensor("wq", [D, DG], bf16, kind="ExternalInput").ap()
    wk = nc.dram_tensor("wk", [D, DG], bf16, kind="ExternalInput").ap()
    wv = nc.dram_tensor("wv", [D, DG], bf16, kind="ExternalInput").ap()
    wo = nc.dram_tensor("wo", [DG, D], bf16, kind="ExternalInput").ap()
    bq = nc.dram_tensor("bq", [P, DG // P], f32, kind="ExternalInput").ap()
    bk = nc.dram_tensor("bk", [P, DG // P], f32, kind="ExternalInput").ap()
    bvr = nc.dram_tensor("bvr", [P, HG, DK], f32, kind="ExternalInput").ap()
    masktri = nc.dram_tensor("masktri", [P, P], bf16, kind="ExternalInput").ap()
    ident = nc.dram_tensor("ident", [P, P], bf16, kind="ExternalInput").ap()
    y = nc.dram_tensor("y", [S, D], bf16, kind="ExternalOutput").ap()

    with tile.TileContext(nc) as tc, ExitStack() as ctx:
        persist = ctx.enter_context(tc.tile_pool(name="persist", bufs=1))
        consts = ctx.enter_context(tc.tile_pool(name="consts", bufs=1))
        xin_pool = ctx.enter_context(tc.tile_pool(name="xin", bufs=6))
        qt_pool = ctx.enter_context(tc.tile_pool(name="qtp", bufs=2))
        ot_pool = ctx.enter_context(tc.tile_pool(name="otpool", bufs=3))
        sb_small = ctx.enter_context(tc.tile_pool(name="sbs", bufs=2))
        outn_pool = ctx.enter_context(tc.tile_pool(name="outn", bufs=2))
        y_pool = ctx.enter_context(tc.tile_pool(name="ysb", bufs=3))
        exp_pool = ctx.enter_context(tc.tile_pool(name="expp", bufs=6))

        # persistent weights / K / V
        wq_t = persist.tile([P, D // P, DG], bf16, tag="wq", name="wq")
        wk_t0 = persist.tile([P, D // P, 256], bf16, tag="wk0", name="wk0")
        wk_t1 = persist.tile([P, D // P, 256], bf16, tag="wk1", name="wk1")
        wv_t = persist.tile([P, D // P, DG], bf16, tag="wv", name="wv")
        wo_t = persist.tile([P, DG // P, D], bf16, tag="wo", name="wo")
        KTs = [[persist.tile([P, 512], bf16, tag=f"KT{m}_{qc}",
                             name=f"KT{m}_{qc}")
                for qc in range(4)] for m in range(4)]
        Vg = persist.tile([P, NKT, HG, DK + 1], bf16, tag="Vg", name="Vg")

        mask_t = consts.tile([P, 2, P], bf16, tag="mask")
        ident_t = consts.tile([P, P], bf16, tag="ident")
        bq_t = consts.tile([P, DG // P], f32, tag="bq")
        bk_t = consts.tile([P, DG // P], f32, tag="bk")
        bvr_t = consts.tile([P, HG, DK], f32, tag="bvr")
        warm_in = consts.tile([1, 4], f32, tag="warmin")
        warm = consts.tile([1, 4], f32, tag="warm")

        # PSUM pools: 4 (scores) + 2 (attnV accum) + 1 (proj) + 1 (yproj)
        sc_ps = ctx.enter_context(tc.tile_pool(name="scps", bufs=SC_BUFS,
                                               space="PSUM"))
        oa_ps = ctx.enter_context(tc.tile_pool(name="oaps", bufs=OA_BUFS,
                                               space="PSUM"))
        pp_ps = ctx.enter_context(tc.tile_pool(name="ppps", bufs=1, space="PSUM"))
        yp_ps = (ctx.enter_context(tc.tile_pool(name="ypps", bufs=1,
                                                space="PSUM"))
                 if SEP_YP else None)

        # prefetch the exp table-set before any real work
        nc.gpsimd.memset(warm_in[:], 0.0)
        nc.scalar.activation(warm[:], warm_in[:], Exp)
        # ones column of V (softmax denominator trick)
        nc.gpsimd.memset(Vg[:, :, :, DK], 1.0)

        def load_xin(src, qb, splits=1):
            xt = xin_pool.tile([P, D // P, 512], bf16, tag="xin")
            step = (D // P) // splits
            for s in range(splits):
                r0, r1 = s * step * P, (s + 1) * step * P
                nc.sync.dma_start(
                    xt[:, s * step:(s + 1) * step, :],
                    src[r0:r1, qb * 512:(qb + 1) * 512]
                    .rearrange("(o p) m -> p o m", p=P))
            return xt

        # alternate proj PSUM banks to avoid WAR chains. With a separate
        # yproj bank, alternate pp/yp everywhere; otherwise alternate with
        # sc banks only while attention is not running (prologue).
        alt_state = [0]
        in_prologue = [True]

        def proj_ps_tile():
            alt_state[0] ^= 1
            if SEP_YP:
                pool, tag = ((pp_ps, "pp"), (yp_ps, "yp"))[alt_state[0]]
            elif in_prologue[0] and alt_state[0]:
                pool, tag = sc_ps, "sc"
            else:
                pool, tag = pp_ps, "pp"
            return pool.tile([P, 512], f32, tag=tag, name=tag)

        def proj_qk_m(w_t, xt, bias_t, dst_tile, m):
            # K weights live in two half tiles (wk_t1 is late-loaded)
            if isinstance(w_t, tuple):
                w_t = w_t[m // 2]
                m = m % 2
            ps = proj_ps_tile()
            for j in range(D // P):
                nc.tensor.matmul(
                    ps[:], w_t[:, j, m * P:(m + 1) * P], xt[:, j, :],
                    start=(j == 0), stop=(j == D // P - 1))
            nc.vector.tensor_scalar_add(
                dst_tile[:], ps[:], bias_t[:, m:m + 1])

        def proj_v_part(xt, qb, mt_l, h2):
            kt = qb * 4 + mt_l
            ps = proj_ps_tile()
            psv = ps[:, 0:DG // 2]
            for j in range(D // P):
                nc.tensor.matmul(
                    psv, xt[:, j, mt_l * P:(mt_l + 1) * P],
                    wv_t[:, j, h2 * 256:(h2 + 1) * 256],
                    start=(j == 0), stop=(j == D // P - 1))
            nc.vector.tensor_add(
                Vg[:, kt, h2 * 4:(h2 + 1) * 4, 0:DK],
                psv.rearrange("p (h d) -> p h d", h=4),
                bvr_t[:, h2 * 4:(h2 + 1) * 4, :])

        ycopy_state = [0]

        def yproj_part(qb, OT, qt_l, nb, ps_pool, ps_tag, split_t=False,
                       tail_ys=None):
            """Emit output-projection part. If split_t, t=0..2 are emitted by
            the caller via the returned closure pair (prefill, finish)."""
            ps = ps_pool.tile([P, 512], f32, tag=ps_tag, name="yp")

            def mm(t):
                nc.tensor.matmul(
                    ps[:], OT[:, t, qt_l * P:(qt_l + 1) * P],
                    wo_t[:, t, nb * 512:(nb + 1) * 512],
                    start=(t == 0), stop=(t == DG // P - 1))

            def finish():
                mm(DG // P - 1)
                if tail_ys is not None:
                    # tail: alternate DVE / ACT so copies pipeline 2-wide
                    # (gpsimd cannot read PSUM)
                    ys_t, idx = tail_ys
                    ycopy_state[0] ^= 1
                    if ycopy_state[0]:
                        nc.vector.tensor_copy(ys_t[:, idx, :], ps[:])
                    else:
                        nc.scalar.activation(
                            ys_t[:, idx, :], ps[:],
                            mybir.ActivationFunctionType.Copy)
                else:
                    ys = y_pool.tile([P, 512], bf16, tag="ys", name="ys")
                    nc.vector.tensor_copy(ys[:], ps[:])
                    nc.sync.dma_start(
                        y[(qb * 4 + qt_l) * P:(qb * 4 + qt_l + 1) * P,
                          nb * 512:(nb + 1) * 512],
                        ys[:])

            def prefill():
                for t in range(DG // P - 1):
                    mm(t)

            if split_t:
                return prefill, finish
            prefill()
            finish()

        def attention_qb(qb, QTcur, fillers, late_fillers=()):
            """fillers: evenly woven closures. late_fillers: (deadline_unit,
            closure) — emitted once done-count reaches deadline (deadlines
            must be achievable: closure emitted before its consumer unit)."""
            nfill0 = len(fillers)
            kmax = 4 * (qb + 1)
            units = [(p, kt, hh)
                     for p in range(HG // 2) for kt in range(kmax)
                     for hh in (0, 1)]
            nunits = len(units)
            done = [0]
            late = list(late_fillers)
            OT = ot_pool.tile([P, HG // 2, 512], bf16, tag="OT", name="OT")
            oas = {}
            tps = {}
            ex_by = {}

            def emit_scores(p, kt, hh):
                diag = kt >= 4 * qb
                f0 = (kt - 4 * qb) * P if diag else 0
                sc = sc_ps.tile([P, 512], f32, tag="sc", name="sc")
                nc.tensor.matmul(
                    sc[:, f0:512],
                    KTs[p][kt // 4][hh * DK:(hh + 1) * DK,
                                    (kt % 4) * P:(kt % 4 + 1) * P],
                    QTcur[p][hh * DK:(hh + 1) * DK, f0:512],
                    start=True, stop=True, tile_position=(hh * DK, 0))
                ex = exp_pool.tile([P, 512], bf16, tag="ex", name="ex")
                nc.scalar.activation(ex[:, f0:512], sc[:, f0:512], Exp)
                if diag:
                    # causal mask, multiplicative post-exp: only the leading
                    # 128-col strip of the valid range is triangular
                    nc.vector.tensor_mul(
                        ex[:, f0:f0 + P], ex[:, f0:f0 + P], mask_t[:, 0:P])
                ex_by[(p, kt, hh)] = ex

            def emit_attnv(p, kt, hh):
                # operand-swapped: attn block [128kt x 128q] is the stationary
                # operand, V [128kt, 65] streams -> only 65 cols per matmul.
                # Output is q-major [128q, 65]; col 64 = softmax denominator.
                if kt == 0:
                    oas[(p, hh)] = oa_ps.tile([P, 4, DK + 1], f32,
                                              tag=f"oa{hh}", name=f"oa{hh}")
                    if hh == 0:
                        tps[p] = None
                oa = oas[(p, hh)]
                ex = ex_by.pop((p, kt, hh))
                # start=True pends-to-zero the WHOLE 2KB bank, so only the
                # first matmul of the bank may carry it: the other s regions
                # initialize via the pending-zero overwrite semantics.
                for s_q in range(4):
                    if kt <= 4 * qb + s_q:
                        nc.tensor.matmul(
                            oa[:, s_q, :], ex[:, s_q * P:(s_q + 1) * P],
                            Vg[:, kt, 2 * p + hh, :],
                            start=(kt == 0 and s_q == 0),
                            stop=(kt == kmax - 1))
                if kt == kmax - 1:
                    rs2 = sb_small.tile([P, 4], f32, tag="rs", name="rs")
                    nc.vector.reciprocal(rs2[:], oa[:, :, DK])
                    outn = outn_pool.tile([P, 4, DK], bf16, tag=f"on{hh}",
                                          name="outn")
                    for s_q in range(4):
                        nc.vector.tensor_scalar_mul(
                            outn[:, s_q, :], oa[:, s_q, 0:DK],
                            rs2[:, s_q:s_q + 1])
                    # transpose back to chan-major into a borrowed yp bank
                    if hh == 0:
                        tps[p] = yp_ps.tile([P, 4, P], bf16, tag="yp",
                                            name="tp")
                    tp = tps[p]
                    for s_q in range(4):
                        nc.tensor.transpose(
                            tp[hh * DK:(hh + 1) * DK, s_q, :],
                            outn[:, s_q, :], ident_t[:])
                    if hh == 1:
                        nc.vector.tensor_copy(
                            OT[:, p, :],
                            tp[:].rearrange("c s q -> c (s q)"))


            def pop_work(i):
                # emit filler work BEFORE attnV(i): it sits between the
                # lookahead scores and the exp-dependent attnV, covering
                # the exp latency in-position
                done[0] = i + 1
                p, kt, hh = units[i]
                nf = 2 if (kt == kmax - 1 and hh == 0
                           and p + 1 < HG // 2) else 0
                while late and late[0][0] <= done[0]:
                    late.pop(0)[1]()
                target_done = (nfill0 * done[0]) // nunits
                while fillers and (nfill0 - len(fillers) < target_done + nf):
                    fillers.pop(0)()
                    if nf:
                        nf -= 1

            dd = min(PIPE_DEPTH, nunits)
            for j in range(dd):
                emit_scores(*units[j])
            while late and late[0][0] <= 0:
                late.pop(0)[1]()
            for i in range(nunits):
                if i + dd < nunits:
                    emit_scores(*units[i + dd])
                pop_work(i)
                emit_attnv(*units[i])
            while late:
                late.pop(0)[1]()
            for f in fillers:
                f()
            del fillers[:]
            return OT

        def proj_block(qb):
            """Emit projections for block qb. Returns (QTcur, fillers,
            late_fillers): late ones run inside attention_qb(qb) itself,
            before their consumer pairs start."""
            fillers = []
            xtv = load_xin(xv, qb)
            for mt_l in range(4):
                for h2 in range(2):
                    fillers.append(
                        lambda mt_l=mt_l, h2=h2, xtv=xtv, qb=qb:
                        proj_v_part(xtv, qb, mt_l, h2))
            QTcur = []
            for m in range(DG // P):
                qt_t = qt_pool.tile([P, 512], bf16, tag=f"QTm{m}",
                                    name=f"QTm{m}")
                QTcur.append(qt_t)
            xtq = load_xin(xq, qb)
            for m in range(2):
                fillers.append(
                    lambda m=m, xtq=xtq: proj_qk_m(
                        wq_t, xtq, bq_t, QTcur[m], m))
            xtk = load_xin(xk, qb)
            for m in range(2):
                fillers.append(
                    lambda m=m, xtk=xtk, qb=qb:
                    proj_qk_m((wk_t0, wk_t1), xtk, bk_t, KTs[m][qb], m))
            # pairs 2 and 3 of block qb are consumed late inside
            # attention_qb(qb): project them there (deadline = unit index
            # safely before first consumer emission p*2*kmax - DEPTH)
            kmax_n = 4 * (qb + 1)
            late = []
            for m in (2, 3):
                dl = max(1, m * kmax_n - PIPE_DEPTH - 2)
                late.append((max(1, dl - 4), lambda m=m, xtq=xtq: proj_qk_m(
                    wq_t, xtq, bq_t, QTcur[m], m)))
                late.append((dl, lambda m=m, xtk=xtk, qb=qb: proj_qk_m(
                    (wk_t0, wk_t1), xtk, bk_t, KTs[m][qb], m)))
            late.sort(key=lambda t: t[0])
            return QTcur, fillers, late

        # ---- block 0 prologue: interleave loads with their first users ----
        nc.sync.dma_start(wk_t0[:],
                          wk[:, 0:256].rearrange("(o p) m -> p o m", p=P))
        xtk0 = load_xin(xk, 0, splits=4)
        nc.sync.dma_start(bk_t[:], bk)
        for m in range(2):
            proj_qk_m((wk_t0, wk_t1), xtk0, bk_t, KTs[m][0], m)
        xtv0 = load_xin(xv, 0, splits=2)
        nc.sync.dma_start(wv_t[:, :, 0:256],
                          wv[:, 0:256].rearrange("(o p) m -> p o m", p=P))
        nc.sync.dma_start(wv_t[:, :, 256:512],
                          wv[:, 256:512].rearrange("(o p) m -> p o m", p=P))
        nc.sync.dma_start(bvr_t[:], bvr)
        for mt_l in range(4):
            for h2 in range(2):
                proj_v_part(xtv0, 0, mt_l, h2)
        xtq0 = load_xin(xq, 0, splits=2)
        nc.sync.dma_start(wq_t[:, :, 0:256],
                          wq[:, 0:256].rearrange("(o p) m -> p o m", p=P))
        nc.sync.dma_start(wq_t[:, :, 256:512],
                          wq[:, 256:512].rearrange("(o p) m -> p o m", p=P))
        nc.sync.dma_start(bq_t[:], bq)
        nc.sync.dma_start(wk_t1[:],
                          wk[:, 256:512].rearrange("(o p) m -> p o m", p=P))
        nc.sync.dma_start(
            mask_t[:], masktri[:, None, :].to_broadcast((P, 2, P)))
        nc.sync.dma_start(ident_t[:], ident)
        QTcur = []
        for m in range(DG // P):
            qt_t = qt_pool.tile([P, 512], bf16, tag=f"QTm{m}", name=f"QTm{m}")
            QTcur.append(qt_t)
        for m in range(2):
            proj_qk_m(wq_t, xtq0, bq_t, QTcur[m], m)
        nc.sync.dma_start(wo_t[:], wo.rearrange("(o p) m -> p o m", p=P))
        # pairs 2,3 of block 0 are projected inside attention_qb(0) itself
        late0 = []
        for m in (2, 3):
            dl = max(1, m * 4 - PIPE_DEPTH - 2)
            late0.append((max(1, dl - 2),
                          lambda m=m: proj_qk_m(wq_t, xtq0, bq_t, QTcur[m], m)))
            late0.append((dl,
                          lambda m=m: proj_qk_m((wk_t0, wk_t1), xtk0, bk_t, KTs[m][0], m)))
        late0.sort(key=lambda t: t[0])

        def yproj_filler(q, O, part_i):
            qt_l, nb = part_i // 2, part_i % 2
            if SEP_YP:
                pool, tag = ((pp_ps, "pp"), (yp_ps, "yp"))[part_i % 2]
            else:
                pool, tag = pp_ps, "pp"
            return (lambda qt_l=qt_l, nb=nb, O=O, q=q, pool=pool, tag=tag:
                    yproj_part(q, O, qt_l, nb, pool, tag))

        in_prologue[0] = False
        prevOT = None     # OT of qb-1
        prev2OT = None    # OT of qb-2 (second half of its yproj deferred)
        pending_late = late0
        for qb in range(NQB):
            last = qb == NQB - 1
            fillers = []
            if prev2OT is not None:
                for part_i in range(2, 8):
                    fillers.append(yproj_filler(qb - 2, prev2OT, part_i))
            if prevOT is not None:
                nparts = 8 if last else 2
                for part_i in range(nparts):
                    fillers.append(yproj_filler(qb - 1, prevOT, part_i))
            nextQT = None
            late = ()
            if not last:
                nextQT, pf, late = proj_block(qb + 1)
                fillers.extend(pf)
            OT = attention_qb(qb, QTcur, fillers, late_fillers=pending_late)
            pending_late = late
            prev2OT, prevOT = prevOT, OT
            QTcur = nextQT

        # ---- tail: output projection of the last block, t-split across 4
        # PSUM banks (yp, pp and two sc rotations are all free now);
        # y stores grouped into two wide DMAs ----
        ys_tail = persist.tile([P, 8, 512], bf16, tag="ystail", name="ystail")
        tailpools = ([(yp_ps, "yp"), (pp_ps, "pp"), (sc_ps, "sc"),
                      (sc_ps, "sc")] if SEP_YP else
                     [(pp_ps, "pp"), (sc_ps, "sc"), (sc_ps, "sc"),
                      (sc_ps, "sc")])
        parts = [(qt_l, nb) for qt_l in range(4) for nb in range(2)]
        pf_fin = []
        for i, (qt_l, nb) in enumerate(parts):
            pool, tag = tailpools[i % 4]
            pf, fin = yproj_part(NQB - 1, prevOT, qt_l, nb, pool, tag,
                                 split_t=True, tail_ys=(ys_tail, i))
            pf_fin.append((pf, fin))
        # prefill t=0..2 of the first 4 parts (only needs pairs 0..2 of OT),
        # then stream finishes; later parts prefill as their bank frees
        for i in range(4):
            pf_fin[i][0]()
        for i in range(len(parts)):
            pf_fin[i][1]()
            if i + 4 < len(parts):
                pf_fin[i + 4][0]()
            if i == 3:
                nc.sync.dma_start(
                    y[(NQB - 1) * 512:(NQB - 1) * 512 + 256, :]
                    .rearrange("(q p) (n m) -> p q n m", p=P, n=2),
                    ys_tail[:, 0:4, :]
                    .rearrange("p (q n) m -> p q n m", q=2))
            if i == 5:
                nc.sync.dma_start(
                    y[(NQB - 1) * 512 + 256:(NQB - 1) * 512 + 384, :]
                    .rearrange("p (n m) -> p n m", n=2),
                    ys_tail[:, 4:6, :])
            if i == 7:
                nc.sync.dma_start(
                    y[(NQB - 1) * 512 + 384:NQB * 512, :]
                    .rearrange("p (n m) -> p n m", n=2),
                    ys_tail[:, 6:8, :])

    nc.compile()
    return nc


def _prep_inputs(query, key, value, Wq, bq, Wk, bk, Wv, bv, Wo, bo):
    scale = 1.0 / np.sqrt(DK)
    qr = _to_bf16(np.asarray(query))
    kr = _to_bf16(np.asarray(key))
    vr = _to_bf16(np.asarray(value))
    wq_full = _to_bf16(np.asarray(Wq).T * scale)   # [D, D], cols = out chans
    wk_full = _to_bf16(np.asarray(Wk).T)
    wv_full = _to_bf16(np.asarray(Wv).T)
    wo_full = _to_bf16(np.asarray(Wo).T)           # [Din, Dout]
    bq_s = np.asarray(bq) * scale

    jj = np.arange(P)[:, None]
    ff = np.arange(P)[None, :]
    masktri = _to_bf16(np.where(jj <= ff, 1.0, 0.0))
    ident = _to_bf16(np.eye(P, dtype=np.float32))

    in_maps = []
    for core in range(8):
        b, hg = core // 2, core % 2
        sl = slice(hg * DG, (hg + 1) * DG)
        in_maps.append({
            "xq": np.ascontiguousarray(qr[b].T),
            "xk": np.ascontiguousarray(kr[b].T),
            "xv": np.ascontiguousarray(vr[b].T),
            "wq": np.ascontiguousarray(wq_full[:, sl]),
            "wk": np.ascontiguousarray(wk_full[:, sl]),
            "wv": np.ascontiguousarray(wv_full[:, sl]),
            "wo": np.ascontiguousarray(wo_full[sl, :]),
            "bq": np.ascontiguousarray(
                bq_s[sl].reshape(DG // P, P).T.astype(np.float32)),
            "bk": np.ascontiguousarray(
                np.asarray(bk)[sl].reshape(DG // P, P).T.astype(np.float32)),
            "bvr": np.broadcast_to(
                np.asarray(bv)[sl].astype(np.float32).reshape(HG, DK),
                (P, HG, DK)).copy(),
            "masktri": masktri,
            "ident": ident,
        })
    return in_maps


def kernel(query, key, value, mask, Wq, bq, Wk, bk, Wv, bv, Wo, bo,
           **run_kwargs):
    from concourse.bass_utils import run_bass_kernel_spmd

    if "nc" not in _cache:
        _cache["nc"] = _build()
    nc = _cache["nc"]

    in_maps = _prep_inputs(query, key, value, Wq, bq, Wk, bk, Wv, bv, Wo, bo)
    res = run_bass_kernel_spmd(nc, in_maps, core_ids=list(range(8)),
                               **run_kwargs)
    bo = np.asarray(bo, dtype=np.float32)
    out = np.empty((B, S, D), dtype=np.float32)
    for b in range(B):
        out[b] = (res.results[2 * b]["y"].astype(np.float32)
                  + res.results[2 * b + 1]["y"].astype(np.float32) + bo)
    _cache["last_results"] = res
    return out
